# revision 9
# baseline (speedup 1.0000x reference)
"""GAT-with-edge-features GNN on 8 Trainium2 NeuronCores.

Strategy (self-contained; shapes hardcoded for the fixed problem size):
  - Relabel nodes so each core owns a contiguous block of NPAD node slots,
    grouped graph-wise by batch_vector (core = graph // 8).
  - Partition edges by the owning core of their dst node; within a core,
    sort edges by dst and FFD-bin-pack whole dst-segments into 128-edge
    tiles (<= 31 segments per tile) so every segment lives in one tile.
  - Every core redundantly computes the full node-level table B (fp16) so
    per-edge h[src] is a row gather from HBM (gpsimd dma_gather).  The
    dst-side values (mean_h_d, s_dst) are NOT gathered per edge: they are
    kept in a slot-ordered table DSTTAB[L] (one row per (tile,seg)) built
    by a 1408-row dma_scatter_add during the previous node phase, then
    broadcast to edges with a per-tile one-hot matmul fused into the same
    PSUM as the edge-feature projection.  Segment softmax is
    denominators-folded: node_out = (sum_e ex*(hs+ep)) / (sum_e ex),
    accumulated per tile with a one-hot segment matmul on the PE.
    Between layers, per-core node states are AllGathered (fp16, Shared).
"""
import sys
import os

for _p in ("/opt/trn_rl_repo", "/root/.axon_site/_ro/trn_rl_repo"):
    if os.path.isdir(_p) and _p not in sys.path:
        sys.path.insert(0, _p)

import numpy as np

# ---------------- problem constants (hardcoded from spec) ----------------
N_NODES = 10000
N_EDGES = 160000
F_IN = 128
E_IN = 32
F = 64
H = 4
OUT = 64
L_MID = 2
G = 64
NEG_SLOPE = 0.2

# ---------------- sharding constants ----------------
C = 8                 # cores
NPAD = 1408           # padded nodes per core (= 11 * 128)
NT_OWN = NPAD // 128  # 11 node tiles per core
NTOT = C * NPAD       # 11264 global padded node slots
ET = 168              # edge tiles per core (FFD-packed; assert fits)
ES = ET * 128         # 21504 edge slots per core
CH = 8                # edge tiles per gather chunk
NCHUNK = ET // CH     # 21
SEG = 32              # segment slots per tile (31 real + 1 trash)
ACCN = ET * SEG       # 5376 accumulation rows
ACC_CONST_MID = ACCN          # const row for zero-degree nodes (mid)
ACC_CONST_OUT = ACCN + 1      # const row (out layer)
ACC_ROWS = ACCN + 64
WB = 384              # B-table row width, layers 0-2
WB3 = 128             # B-table row width, layer 3
WA = 260              # accum row content width, mid (256 msg + 4 ex)
WA3 = 65              # accum row content width, out (64 msg + 1 ex)

_PROGRAM_CACHE = {}


def _wrap16(u):
    """int index vector [n] -> dma_gather idx layout [128, n//16] int16."""
    n = len(u)
    assert n % 16 == 0
    a = np.asarray(u).reshape(n // 16, 16).T
    return np.ascontiguousarray(np.tile(a, (8, 1)).astype(np.int16))


def _make_wn_aug(Wn, a):
    """Wn [fin,H,F], a [3,H,F] -> augmented node weight [fin, WB or WB3]."""
    fin, Hh, Ff = Wn.shape
    w = WB if Hh == 4 else WB3
    Wa = np.zeros((fin, w), np.float32)
    if Hh == 4:
        Wa[:, 0:256] = Wn.reshape(fin, 256)
        Wa[:, 256:320] = Wn.mean(axis=1)
        Wa[:, 320:324] = np.einsum("ihf,hf->ih", Wn, a[0])
    else:
        Wa[:, 0:64] = Wn[:, 0, :]
        Wa[:, 64] = np.einsum("if,f->i", Wn[:, 0, :], a[0, 0])
    return Wa


def _make_we_aug(We, a):
    fin, Hh, Ff = We.shape
    w = WB if Hh == 4 else WB3
    Wa = np.zeros((fin, w), np.float32)
    if Hh == 4:
        Wa[:, 0:256] = We.reshape(fin, 256)
        Wa[:, 256:320] = We.mean(axis=1)
        Wa[:, 320:324] = np.einsum("ihf,hf->ih", We, a[2])
    else:
        Wa[:, 0:64] = We[:, 0, :]
        Wa[:, 64] = np.einsum("if,f->i", We[:, 0, :], a[2, 0])
    return Wa


def _make_wdst(Wn, a):
    """dst-side row projector: x_d -> [mean_h_d(0:64) | s_dst(64:64+H) | 0]."""
    fin, Hh, Ff = Wn.shape
    Wa = np.zeros((fin, 128), np.float32)
    if Hh == 4:
        Wa[:, 0:64] = Wn.mean(axis=1)
        Wa[:, 64:68] = np.einsum("ihf,hf->ih", Wn, a[1])
    else:
        # out layer: edge_out unused, only the score is needed
        Wa[:, 64] = np.einsum("if,f->i", Wn[:, 0, :], a[1, 0])
    return Wa


def _f16(x):
    return np.ascontiguousarray(np.asarray(x, np.float32).astype(np.float16))


def _pack_core(dst_local_sorted):
    """FFD-pack whole dst-segments into tiles of <=128 edges, <=31 segments.

    dst_local_sorted: dst node (orig id) per edge, sorted ascending.
    Returns (tile_of_seg, segidx_of_seg, uniq, start, cnts, tile_edges)
    where seg i is the i-th unique dst in ascending-dst order.
    """
    uniq, start, cnts = np.unique(dst_local_sorted, return_index=True,
                                  return_counts=True)
    nseg = len(uniq)
    order = np.argsort(-cnts, kind="stable")   # big segments first
    tile_edges = []
    tile_nseg = []
    tile_of_seg = np.empty(nseg, np.int64)
    segidx_of_seg = np.empty(nseg, np.int64)
    for si in order:
        cnt = cnts[si]
        placed = False
        for t in range(len(tile_edges)):
            if tile_edges[t] + cnt <= 128 and tile_nseg[t] < SEG - 1:
                tile_of_seg[si] = t
                segidx_of_seg[si] = tile_nseg[t]
                tile_edges[t] += cnt
                tile_nseg[t] += 1
                placed = True
                break
        if not placed:
            tile_of_seg[si] = len(tile_edges)
            segidx_of_seg[si] = 0
            tile_edges.append(int(cnt))
            tile_nseg.append(1)
    assert len(tile_edges) <= ET, len(tile_edges)
    return tile_of_seg, segidx_of_seg, uniq, start, cnts, tile_edges


def _preprocess(inputs):
    """Host-side: relabel nodes, pack edges, build per-core device inputs."""
    nf = np.asarray(inputs["node_features"], np.float32)
    ef = np.asarray(inputs["edge_features"], np.float32)
    ei = np.asarray(inputs["edge_index"], np.int64)
    bv = np.asarray(inputs["batch_vector"], np.int64)

    core_of_node = bv // (G // C)
    core_of_edge = core_of_node[ei[1]]
    cnt_graph = np.bincount(bv, minlength=G).astype(np.float32)

    # ---- per-core packing (uses orig ids), then global relabel
    packs = []
    new_id = np.empty(N_NODES, np.int64)
    core_node_lists = []
    for c in range(C):
        eids = np.nonzero(core_of_edge == c)[0]
        eo = eids[np.argsort(ei[1][eids], kind="stable")]
        dsts = ei[1][eo]
        tile_of_seg, segidx_of_seg, uniq, start, cnts, tile_edges = \
            _pack_core(dsts)
        nseg = len(uniq)

        ntile = len(tile_edges)
        seg_order = np.lexsort((segidx_of_seg, tile_of_seg))
        off_in_tile = np.zeros(nseg, np.int64)
        fill = np.zeros(ntile, np.int64)
        for si in seg_order:
            t = tile_of_seg[si]
            off_in_tile[si] = fill[t]
            fill[t] += cnts[si]

        slot_of_edge = np.empty(len(eo), np.int64)
        seg_of_edge = np.empty(len(eo), np.int64)
        for si in range(nseg):
            s0 = start[si]
            cnt = cnts[si]
            slot_of_edge[s0:s0 + cnt] = (tile_of_seg[si] * 128
                                         + off_in_tile[si] + np.arange(cnt))
            seg_of_edge[s0:s0 + cnt] = segidx_of_seg[si]

        # node order: nodes with segments by (tile, segidx), then the rest
        nodes_with_seg = uniq[seg_order]
        own = np.nonzero(core_of_node == c)[0]
        rest = own[~np.isin(own, nodes_with_seg)]
        ordered = np.concatenate([nodes_with_seg, rest])
        assert len(ordered) <= NPAD, len(ordered)
        new_id[ordered] = c * NPAD + np.arange(len(ordered))
        core_node_lists.append(ordered)
        packs.append(dict(eo=eo, slot_of_edge=slot_of_edge,
                          seg_of_edge=seg_of_edge, nseg=nseg,
                          tile_of_seg=tile_of_seg, segidx_of_seg=segidx_of_seg,
                          seg_order=seg_order))

    x0T = np.zeros((F_IN, NTOT), np.float32)
    x0T[:, new_id] = nf.T
    src_new = new_id[ei[0]]

    per_core = []
    for c in range(C):
        p = packs[c]
        eo, slot_of_edge, seg_of_edge = p["eo"], p["slot_of_edge"], p["seg_of_edge"]
        nseg = p["nseg"]

        slot_src = np.zeros(ES, np.int64)
        slot_src[slot_of_edge] = src_new[eo]
        seg_of_slot = np.full(ES, SEG - 1, np.int64)
        seg_of_slot[slot_of_edge] = seg_of_edge

        # one-hot segment matrices (static): soh [128, ET*SEG], sohT [SEG, ET*128]
        soh = np.zeros((128, ET * SEG), np.float32)
        sohT = np.zeros((SEG, ET * 128), np.float32)
        sl = np.arange(ES)
        soh[sl % 128, (sl // 128) * SEG + seg_of_slot] = 1.0
        sohT[seg_of_slot, sl] = 1.0

        # node k (k-th in core order) -> ACC row / DSTTAB slot
        accslot = np.full(NPAD, -1, np.int64)
        so = p["seg_order"]
        accslot[0:nseg] = p["tile_of_seg"][so] * SEG + p["segidx_of_seg"][so]
        pos_m = accslot.copy()
        pos_m[pos_m < 0] = ACC_CONST_MID
        pos_o = accslot.copy()
        pos_o[pos_o < 0] = ACC_CONST_OUT
        dstscat = accslot.copy()
        dstscat[dstscat < 0] = ACCN       # trash row, never read back

        e0T_c = np.zeros((E_IN, ES), np.float32)
        e0T_c[:, slot_of_edge] = ef[eo].T

        # readout one-hot (scaled by 1/count) for the 8 local graphs
        g1h = np.zeros((128, NT_OWN * 8), np.float32)
        ids = core_node_lists[c]
        if len(ids):
            j = np.arange(len(ids))
            gl = bv[ids] % 8
            vals = 1.0 / np.maximum(cnt_graph[bv[ids]], 1.0)
            g1h[j % 128, (j // 128) * 8 + gl] = vals

        per_core.append(dict(
            e0T=_f16(e0T_c),
            srcidx=_wrap16(slot_src),
            soh=_f16(soh),
            sohT=_f16(sohT),
            accidx_m=_wrap16(pos_m),
            accidx_o=_wrap16(pos_o),
            dstscat=_wrap16(dstscat),
            g1h=g1h,
            x0own=_f16(x0T[:, c * NPAD:(c + 1) * NPAD]),
        ))

    # ---- weights
    Wn0 = np.asarray(inputs["Wn0"], np.float32)
    We0 = np.asarray(inputs["We0"], np.float32)
    a0 = np.asarray(inputs["a0"], np.float32)
    Wnm = np.asarray(inputs["Wn_mid"], np.float32)
    Wem = np.asarray(inputs["We_mid"], np.float32)
    am = np.asarray(inputs["a_mid"], np.float32)
    Wno = np.asarray(inputs["Wn_out"], np.float32)
    Weo = np.asarray(inputs["We_out"], np.float32)
    ao = np.asarray(inputs["a_out"], np.float32)

    shared = dict(
        x0T=_f16(x0T),
        accconst=np.zeros((2, 320), np.float32),
        wn0=_f16(_make_wn_aug(Wn0, a0)),
        we0=_f16(_make_we_aug(We0, a0)),
        wn1=_f16(_make_wn_aug(Wnm[0], am[0])),
        we1=_f16(_make_we_aug(Wem[0], am[0])),
        wn2=_f16(_make_wn_aug(Wnm[1], am[1])),
        we2=_f16(_make_we_aug(Wem[1], am[1])),
        wn3=_f16(_make_wn_aug(Wno, ao)),
        we3=_f16(_make_we_aug(Weo, ao)),
        wd0=_f16(_make_wdst(Wn0, a0)),
        wd1=_f16(_make_wdst(Wnm[0], am[0])),
        wd2=_f16(_make_wdst(Wnm[1], am[1])),
        wd3=_f16(_make_wdst(Wno, ao)),
    )
    shared["accconst"][0, 256:260] = 1.0   # mid const row: num=0, den=1
    shared["accconst"][1, 64] = 1.0        # out const row

    in_maps = []
    for c in range(C):
        m = dict(shared)
        m.update(per_core[c])
        in_maps.append({k: np.ascontiguousarray(v) for k, v in m.items()})
    return in_maps


def _build_program():
    from concourse import bacc, mybir, tile
    from concourse.masks import make_identity

    f32 = mybir.dt.float32
    f16 = mybir.dt.float16
    i16 = mybir.dt.int16
    AOP = mybir.AluOpType
    AF = mybir.ActivationFunctionType

    nc = bacc.Bacc("TRN2", target_bir_lowering=False, debug=False, num_devices=C)

    din = {}
    def dt(name, shape, dtype=f32, kind="ExternalInput"):
        din[name] = nc.dram_tensor(name, shape, dtype, kind=kind)
        return din[name]

    dt("x0T", [F_IN, NTOT], f16)
    dt("x0own", [F_IN, NPAD], f16)
    dt("e0T", [E_IN, ES], f16)
    dt("accconst", [2, 320])
    dt("srcidx", [128, ES // 16], i16)
    dt("soh", [128, ET * SEG], f16)
    dt("sohT", [SEG, ET * 128], f16)
    dt("accidx_m", [128, NPAD // 16], i16)
    dt("accidx_o", [128, NPAD // 16], i16)
    dt("dstscat", [128, NPAD // 16], i16)
    dt("g1h", [128, NT_OWN * 8])
    dt("wn0", [F_IN, WB], f16)
    dt("we0", [E_IN, WB], f16)
    for k in (1, 2):
        dt(f"wn{k}", [F, WB], f16)
        dt(f"we{k}", [F, WB], f16)
    dt("wn3", [F, WB3], f16)
    dt("we3", [F, WB3], f16)
    dt("wd0", [F_IN, 128], f16)
    for k in (1, 2, 3):
        dt(f"wd{k}", [F, 128], f16)
    out_t = dt("out", [8, OUT], kind="ExternalOutput")

    with tile.TileContext(nc) as tc:
        with tc.tile_pool(name="persist", bufs=1) as pp, \
             tc.tile_pool(name="work", bufs=2) as wp, \
             tc.tile_pool(name="work3", bufs=3) as wp3, \
             tc.tile_pool(name="pmain", bufs=3, space="PSUM") as pmain, \
             tc.tile_pool(name="pseg", bufs=2, space="PSUM") as pseg, \
             tc.tile_pool(name="ptr", bufs=2, space="PSUM") as ptr, \
             tc.tile_pool(name="pg", bufs=1, space="PSUM") as pgp, \
             tc.tile_pool(name="dram", bufs=1, space="DRAM") as dp:

            # ---- persistent SBUF
            def load_persist(name, shape, dtype=f32):
                t = pp.tile(shape, dtype, tag=name)
                nc.sync.dma_start(out=t[:], in_=din[name].ap())
                return t

            srcidx = load_persist("srcidx", [128, ES // 16], i16)
            soh = load_persist("soh", [128, ET * SEG], f16)
            sohT = load_persist("sohT", [SEG, ET * 128], f16)
            accidx_m = load_persist("accidx_m", [128, NPAD // 16], i16)
            accidx_o = load_persist("accidx_o", [128, NPAD // 16], i16)
            dstscat = load_persist("dstscat", [128, NPAD // 16], i16)
            g1h = load_persist("g1h", [128, NT_OWN * 8])
            x0own = load_persist("x0own", [F_IN, NPAD], f16)
            wn = [load_persist("wn0", [F_IN, WB], f16),
                  load_persist("wn1", [F, WB], f16),
                  load_persist("wn2", [F, WB], f16),
                  load_persist("wn3", [F, WB3], f16)]
            we = [load_persist("we0", [E_IN, WB], f16),
                  load_persist("we1", [F, WB], f16),
                  load_persist("we2", [F, WB], f16),
                  load_persist("we3", [F, WB3], f16)]
            wd = [load_persist("wd0", [F_IN, 128], f16),
                  load_persist("wd1", [F, 128], f16),
                  load_persist("wd2", [F, 128], f16),
                  load_persist("wd3", [F, 128], f16)]
            ident = pp.tile([128, 128], f16, tag="ident")
            make_identity(nc, ident[:])

            # ---- DRAM scratch
            B = [dp.tile([NTOT, WB], f16, tag="B0", name="B0"),
                 dp.tile([NTOT, WB], f16, tag="B1", name="B1"),
                 dp.tile([NTOT, WB], f16, tag="B2", name="B2"),
                 dp.tile([NTOT, WB3], f16, tag="B3", name="B3")]
            ACCD = dp.tile([ACC_ROWS, 320], f32, tag="ACC")
            EA = dp.tile([F, ES], f16, tag="EA")
            EB = dp.tile([F, ES], f16, tag="EB")
            AGIN = dp.tile([F, NPAD], f16, tag="AGIN")
            CCT = [dp.tile([C, F, NPAD], f16, tag=f"CC{k}", name=f"CC{k}",
                           addr_space="Shared") for k in range(3)]
            DTB = [dp.tile([ACCN + 64, 128], f16, tag=f"DT{k}", name=f"DT{k}")
                   for k in range(4)]

            # const rows + zero-init of DSTTABs
            nc.sync.dma_start(out=ACCD[ACC_CONST_MID:ACC_CONST_MID + 2, :],
                              in_=din["accconst"].ap())
            zeros = pp.tile([128, ACCN], f16, tag="zeros")
            nc.vector.memset(zeros[:], 0.0)
            for k in range(4):
                nc.sync.dma_start(
                    out=DTB[k][0:ACCN, :].rearrange("(a p) w -> p a w", p=128),
                    in_=zeros[:].rearrange("p (a w) -> p a w", w=128))

            # ------------------------------------------------------------------
            def build_dst0():
                """DSTTAB[0] rows from raw node features (own block)."""
                dstg = wp.tile([128, NT_OWN, 128], f16, tag="dstg", bufs=1,
                               name="dstg0")
                for nt in range(NT_OWN):
                    pd = pmain.tile([128, WB], f32, space="PSUM", tag="pmain")
                    nc.tensor.matmul(out=pd[:, 0:128],
                                     lhsT=x0own[:, nt * 128:(nt + 1) * 128],
                                     rhs=wd[0][:, :], start=True, stop=True)
                    if nt % 2 == 0:
                        nc.vector.tensor_copy(out=dstg[:, nt, :], in_=pd[:, 0:128])
                    else:
                        nc.scalar.copy(out=dstg[:, nt, :], in_=pd[:, 0:128])
                nc.gpsimd.dma_scatter_add(
                    out_ap=DTB[0][:, :], in_ap=dstg[:], idxs_ap=dstscat[:],
                    num_idxs=NPAD, num_idxs_reg=NPAD, elem_size=128)

            # ------------------------------------------------------------------
            def build_B(L):
                """Full-table node matmul: B[L] rows for all NTOT nodes."""
                WBL = WB if L < 3 else WB3
                for blk in range(C):
                    if L == 0:
                        xb = wp.tile([F_IN, NPAD], f16, tag="xblk")
                        nc.sync.dma_start(out=xb[:], in_=din["x0T"].ap()[:, blk * NPAD:(blk + 1) * NPAD])
                        kn = F_IN
                    else:
                        xb = wp.tile([F, NPAD], f16, tag="xblk", name="xblkm")
                        nc.sync.dma_start(out=xb[:], in_=CCT[L - 1][blk, :, :])
                        kn = F
                    for j0 in range(0, NT_OWN, 4):
                        nb = min(4, NT_OWN - j0)
                        bs = wp.tile([128, 4, WB], f16, tag="bstage")
                        for dj in range(nb):
                            j = j0 + dj
                            pb = pmain.tile([128, WB], f32, space="PSUM", tag="pmain")
                            nc.tensor.matmul(out=pb[:, 0:WBL],
                                             lhsT=xb[:kn, j * 128:(j + 1) * 128],
                                             rhs=wn[L][:kn, 0:WBL],
                                             start=True, stop=True)
                            if dj % 2 == 0:
                                nc.vector.tensor_copy(out=bs[:, dj, 0:WBL], in_=pb[:, 0:WBL])
                            else:
                                nc.scalar.copy(out=bs[:, dj, 0:WBL], in_=pb[:, 0:WBL])
                        nt = blk * NT_OWN + j0
                        nc.sync.dma_start(
                            out=B[L][nt * 128:(nt + nb) * 128, 0:WBL].rearrange(
                                "(b p) w -> p b w", p=128),
                            in_=bs[:, 0:nb, 0:WBL])

            # ------------------------------------------------------------------
            def edge_phase(L):
                mid = L < 3
                WBL = WB if mid else WB3
                WE = 324 if mid else 128      # cols actually written in pm
                Hk = H if mid else 1
                MS = 256 if mid else 64       # msg cols
                WAk = WA if mid else WA3      # acc row content
                SS = 320 if mid else 64       # col of summed scores
                DOFF = 256 if mid else 0      # psum col where dst-mm lands
                DW = 68 if mid else 128       # dst row cols used
                exscale = float(Hk)
                esrc = [None, EA, EB, EA][L]
                edst = [EA, EB, EA, None][L]

                for c in range(NCHUNK):
                    cs, ce = c * CH * 128, (c + 1) * CH * 128
                    gb = wp.tile([128, CH, WBL], f16, tag="gb", name="gb")
                    nc.gpsimd.dma_gather(
                        out_ap=gb[:], in_ap=B[L][:, :],
                        idxs_ap=srcidx[:, c * (CH * 8):(c + 1) * (CH * 8)],
                        num_idxs=CH * 128, num_idxs_reg=CH * 128, elem_size=WBL)
                    dr = wp.tile([SEG, CH, 128], f16, tag="dr", name="dr")
                    nc.sync.dma_start(
                        out=dr[:],
                        in_=DTB[L][c * CH * SEG:(c + 1) * CH * SEG, :].rearrange(
                            "(t s) w -> s t w", s=SEG))
                    if L == 0:
                        ech = wp.tile([E_IN, CH * 128], f16, tag="ech0")
                        nc.sync.dma_start(out=ech[:], in_=din["e0T"].ap()[:, cs:ce])
                        ke = E_IN
                    else:
                        ech = wp.tile([F, CH * 128], f16, tag="ech")
                        nc.sync.dma_start(out=ech[:], in_=esrc[0:F, cs:ce])
                        ke = F

                    mwm = wp.tile([128, CH, 256], f16, tag="mwm")
                    lr = wp3.tile([128, CH, H], f16, tag="lr")
                    ex = wp3.tile([128, CH, H], f16, tag="ex")
                    if mid:
                        eech = wp.tile([128, CH, F], f16, tag="eech")
                        rrch = wp.tile([128, CH, F], f16, tag="rrch")

                    for tl in range(CH):
                        t = c * CH + tl
                        pm = pmain.tile([128, WB], f32, space="PSUM", tag="pmain")
                        nc.tensor.matmul(out=pm[:, 0:WE],
                                         lhsT=ech[:ke, tl * 128:(tl + 1) * 128],
                                         rhs=we[L][:ke, 0:WE], start=True,
                                         stop=False, skip_group_check=True)
                        nc.tensor.matmul(out=pm[:, 0:WE], lhsT=ident[:],
                                         rhs=gb[:, tl, 0:WE], start=False,
                                         stop=False, skip_group_check=True)
                        nc.tensor.matmul(out=pm[:, DOFF:DOFF + DW],
                                         lhsT=sohT[:, t * 128:(t + 1) * 128],
                                         rhs=dr[:, tl, 0:DW], start=False,
                                         stop=True, skip_group_check=True)
                        # score path: ex = exp(leaky_relu(s_src+s_dst+s_e))
                        nc.scalar.activation(lr[:, tl, 0:Hk], pm[:, SS:SS + Hk],
                                             AF.Lrelu, alpha=NEG_SLOPE)
                        nc.scalar.activation(ex[:, tl, 0:Hk], lr[:, tl, 0:Hk], AF.Exp)
                        if mid:
                            nc.scalar.activation(eech[:, tl, :], pm[:, 256:320], AF.Exp)
                            nc.vector.tensor_scalar(out=rrch[:, tl, :],
                                                    in0=pm[:, 256:320], scalar1=0.0,
                                                    scalar2=None, op0=AOP.max)
                        nc.vector.tensor_tensor(
                            out=mwm[:, tl, 0:MS].rearrange("p (h f) -> p h f", h=Hk),
                            in0=pm[:, 0:MS].rearrange("p (h f) -> p h f", h=Hk),
                            in1=ex[:, tl, 0:Hk].unsqueeze(2).to_broadcast([128, Hk, F]),
                            op=AOP.mult)

                    # ---- batched small ops over the whole chunk
                    if mid:
                        denw = wp3.tile([128, CH, H], f16, tag="denw")
                        nc.vector.tensor_scalar_mul(denw[:], ex[:], exscale)
                        eem = wp.tile([128, CH, F], f16, tag="eem")
                        nc.vector.tensor_scalar(out=eem[:], in0=eech[:], scalar1=-1.0,
                                                scalar2=None, op0=AOP.add)
                        ench = wp.tile([128, CH, F], f16, tag="ench")
                        nc.vector.tensor_tensor(out=ench[:], in0=eem[:], in1=rrch[:],
                                                op=AOP.min)
                        est = wp.tile([128, CH // 2, 128], f16, tag="est")
                    else:
                        denw = ex

                    acst = wp.tile([64, 4, WA], f32, tag="acst")
                    for g in range(4):
                        ps = pseg.tile([64, WA], f32, space="PSUM", tag="pseg")
                        for q in range(2):
                            tl = g * 2 + q
                            t = c * CH + tl
                            nc.tensor.matmul(out=ps[q * SEG:(q + 1) * SEG, 0:MS],
                                             lhsT=soh[:, t * SEG:(t + 1) * SEG],
                                             rhs=mwm[:, tl, 0:MS], start=True,
                                             stop=False, skip_group_check=True)
                            nc.tensor.matmul(out=ps[q * SEG:(q + 1) * SEG, MS:MS + Hk],
                                             lhsT=soh[:, t * SEG:(t + 1) * SEG],
                                             rhs=denw[:, tl, 0:Hk], start=False,
                                             stop=True, skip_group_check=True)
                        if g % 2 == 0:
                            nc.vector.tensor_copy(out=acst[:, g, 0:WAk], in_=ps[:, 0:WAk])
                        else:
                            nc.scalar.copy(out=acst[:, g, 0:WAk], in_=ps[:, 0:WAk])
                    if mid:
                        for pr2 in range(CH // 2):
                            pt = ptr.tile([128, 128], f16, space="PSUM", tag="ptr")
                            nc.tensor.transpose(out=pt[0:F, :], in_=ench[:, 2 * pr2, :],
                                                identity=ident[:])
                            nc.tensor.transpose(out=pt[F:128, :], in_=ench[:, 2 * pr2 + 1, :],
                                                identity=ident[:])
                            nc.scalar.copy(out=est[:, pr2, :], in_=pt[:])
                    nc.sync.dma_start(
                        out=ACCD[c * CH * SEG:(c + 1) * CH * SEG, 0:WAk].rearrange(
                            "(g p) w -> p g w", p=64),
                        in_=acst[:, :, 0:WAk])
                    if mid:
                        nc.sync.dma_start(
                            out=edst[0:F, cs:ce].rearrange(
                                "f (i j p) -> f j i p", j=2, p=128)[:, 0, :, :],
                            in_=est[0:F, :, :])
                        nc.sync.dma_start(
                            out=edst[0:F, cs:ce].rearrange(
                                "f (i j p) -> f j i p", j=2, p=128)[:, 1, :, :],
                            in_=est[F:128, :, :])

            # ------------------------------------------------------------------
            def node_phase(L):
                if L < 3:
                    gn = wp.tile([128, NT_OWN, 320], f32, tag="gn", bufs=1)
                    nc.gpsimd.dma_gather(
                        out_ap=gn[:, 0:8, :], in_ap=ACCD[:, :], idxs_ap=accidx_m[:, 0:64],
                        num_idxs=1024, num_idxs_reg=1024, elem_size=320)
                    nc.gpsimd.dma_gather(
                        out_ap=gn[:, 8:NT_OWN, :], in_ap=ACCD[:, :], idxs_ap=accidx_m[:, 64:88],
                        num_idxs=NPAD - 1024, num_idxs_reg=NPAD - 1024, elem_size=320)
                    xstg = wp.tile([F, NT_OWN, 128], f16, tag="xstg", bufs=1)
                    dstg = wp.tile([128, NT_OWN, 128], f16, tag="dstg", bufs=1)
                    for nt in range(NT_OWN):
                        rec = wp3.tile([128, H], f32, tag="rec")
                        nc.vector.reciprocal(out=rec[:], in_=gn[:, nt, 256:260])
                        pr = wp3.tile([128, 256], f32, tag="pr")
                        nc.vector.tensor_tensor(
                            out=pr[:].rearrange("p (h f) -> p h f", h=H),
                            in0=gn[:, nt, 0:256].rearrange("p (h f) -> p h f", h=H),
                            in1=rec[:].unsqueeze(2).to_broadcast([128, H, F]),
                            op=AOP.mult)
                        xo = wp3.tile([128, F], f32, tag="xo")
                        nc.vector.tensor_reduce(
                            out=xo[:], in_=pr[:].rearrange("p (h f) -> p f h", h=H),
                            axis=mybir.AxisListType.X, op=AOP.add)
                        # ELU(x) = min(exp(x)-1, max(x, 0))
                        xe = wp3.tile([128, F], f16, tag="ee")
                        nc.scalar.activation(xe[:], xo[:], AF.Exp)
                        xem = wp3.tile([128, F], f16, tag="eem1")
                        nc.vector.tensor_scalar(out=xem[:], in0=xe[:], scalar1=-1.0,
                                                scalar2=None, op0=AOP.add)
                        xr = wp3.tile([128, F], f16, tag="rr")
                        nc.vector.tensor_scalar(out=xr[:], in0=xo[:], scalar1=0.0,
                                                scalar2=None, op0=AOP.max)
                        xs = wp3.tile([128, F], f16, tag="en")
                        nc.vector.tensor_tensor(out=xs[:], in0=xem[:], in1=xr[:], op=AOP.min)
                        pt = ptr.tile([F, 128], f16, space="PSUM", tag="ptr")
                        nc.tensor.transpose(out=pt[:], in_=xs[:], identity=ident[:])
                        if nt % 2 == 0:
                            nc.vector.tensor_copy(out=xstg[:, nt, :], in_=pt[:])
                        else:
                            nc.scalar.copy(out=xstg[:, nt, :], in_=pt[:])
                        # next layer's dst-table rows (node-major)
                        pd = pmain.tile([128, WB], f32, space="PSUM", tag="pmain")
                        nc.tensor.matmul(out=pd[:, 0:128],
                                         lhsT=xstg[:, nt, :], rhs=wd[L + 1][:, :],
                                         start=True, stop=True)
                        if nt % 2 == 0:
                            nc.scalar.copy(out=dstg[:, nt, :], in_=pd[:, 0:128])
                        else:
                            nc.vector.tensor_copy(out=dstg[:, nt, :], in_=pd[:, 0:128])
                    nc.sync.dma_start(out=AGIN[:, :].rearrange("f (t p) -> f t p", p=128),
                                      in_=xstg[:])
                    nc.gpsimd.dma_scatter_add(
                        out_ap=DTB[L + 1][:, :], in_ap=dstg[:], idxs_ap=dstscat[:],
                        num_idxs=NPAD, num_idxs_reg=NPAD, elem_size=128)
                    nc.gpsimd.collective_compute(
                        "AllGather", AOP.bypass,
                        replica_groups=[list(range(C))],
                        ins=[AGIN[:]], outs=[CCT[L][:]])
                    build_B(L + 1)
                else:
                    gn = wp.tile([128, NT_OWN, 128], f32, tag="gn", name="gn3", bufs=1)
                    nc.gpsimd.dma_gather(
                        out_ap=gn[:, 0:8, :], in_ap=ACCD[:, 0:128], idxs_ap=accidx_o[:, 0:64],
                        num_idxs=1024, num_idxs_reg=1024, elem_size=128, elem_step=320)
                    nc.gpsimd.dma_gather(
                        out_ap=gn[:, 8:NT_OWN, :], in_ap=ACCD[:, 0:128], idxs_ap=accidx_o[:, 64:88],
                        num_idxs=NPAD - 1024, num_idxs_reg=NPAD - 1024, elem_size=128, elem_step=320)
                    pg = pgp.tile([8, OUT], f32, space="PSUM", tag="pg")
                    for nt in range(NT_OWN):
                        rec = wp3.tile([128, H], f32, tag="rec")
                        nc.vector.reciprocal(out=rec[:, 0:1], in_=gn[:, nt, 64:65])
                        nod = wp3.tile([128, 256], f32, tag="pr", name="nod")
                        nc.vector.tensor_tensor(out=nod[:, 0:OUT], in0=gn[:, nt, 0:OUT],
                                                in1=rec[:, 0:1].to_broadcast([128, OUT]),
                                                op=AOP.mult)
                        nc.tensor.matmul(out=pg[:], lhsT=g1h[:, nt * 8:(nt + 1) * 8],
                                         rhs=nod[:, 0:OUT], start=(nt == 0),
                                         stop=(nt == NT_OWN - 1), skip_group_check=True)
                    og = wp3.tile([8, OUT], f32, tag="og")
                    nc.vector.tensor_copy(out=og[:], in_=pg[:])
                    nc.sync.dma_start(out=out_t.ap(), in_=og[:])

            # ------------------------------------------------------------------
            stage = os.environ.get("KERNEL_STAGE", "full")
            og0 = wp3.tile([8, OUT], f32, tag="og", name="og0")
            nc.vector.memset(og0[:], 0.0)
            nc.sync.dma_start(out=out_t.ap(), in_=og0[:])
            if stage == "full":
                build_dst0()
                build_B(0)
                for L in range(4):
                    edge_phase(L)
                    node_phase(L)
            else:
                n = int(stage)  # 1=B0+dst0, 2=+edge0, 3=+node0, 4=+edge1, ...
                step = 0
                build_dst0()
                build_B(0)
                step += 1
                for L in range(4):
                    if step >= n:
                        break
                    edge_phase(L)
                    step += 1
                    if step >= n:
                        break
                    node_phase(L)
                    step += 1

    nc.compile()
    return nc


def _get_program():
    if "nc" not in _PROGRAM_CACHE:
        _PROGRAM_CACHE["nc"] = _build_program()
    return _PROGRAM_CACHE["nc"]


def kernel(**inputs):
    from concourse.bass_utils import run_bass_kernel_spmd

    nc = _get_program()
    in_maps = _preprocess(inputs)
    trace = bool(int(os.environ.get("KERNEL_TRACE", "0")))
    res = run_bass_kernel_spmd(nc, in_maps, core_ids=list(range(C)), trace=trace)
    _PROGRAM_CACHE["last_result"] = res
    out = np.concatenate([np.asarray(res.results[c]["out"]) for c in range(C)], axis=0)
    return out.astype(np.float32)


# revision 10
# speedup vs baseline: 1.4682x; 1.4682x over previous
"""GAT-with-edge-features GNN on 8 Trainium2 NeuronCores.

Strategy (self-contained; shapes hardcoded for the fixed problem size):
  - Relabel nodes so each core owns a contiguous block of NPAD node slots,
    grouped graph-wise by batch_vector (core = graph // 8).
  - Partition edges by the owning core of their dst node; within a core,
    sort edges by dst and FFD-bin-pack whole dst-segments into 128-edge
    tiles (<= 31 segments per tile) so every segment lives in one tile.
  - Every core redundantly computes the full node-level table B (fp16) so
    per-edge h[src] is a row gather from HBM (gpsimd dma_gather).  The
    dst-side values (mean_h_d, s_dst) are NOT gathered per edge: they are
    kept in a slot-ordered table DSTTAB[L] (one row per (tile,seg)) built
    by a 1408-row dma_scatter_add during the previous node phase, then
    broadcast to edges with a per-tile one-hot matmul fused into the same
    PSUM as the edge-feature projection.  Segment softmax is
    denominators-folded: node_out = (sum_e ex*(hs+ep)) / (sum_e ex),
    accumulated per tile with a one-hot segment matmul on the PE.
    Between layers, per-core node states are AllGathered (fp16, Shared).
"""
import sys
import os

for _p in ("/opt/trn_rl_repo", "/root/.axon_site/_ro/trn_rl_repo"):
    if os.path.isdir(_p) and _p not in sys.path:
        sys.path.insert(0, _p)

import numpy as np

# ---------------- problem constants (hardcoded from spec) ----------------
N_NODES = 10000
N_EDGES = 160000
F_IN = 128
E_IN = 32
F = 64
H = 4
OUT = 64
L_MID = 2
G = 64
NEG_SLOPE = 0.2

# ---------------- sharding constants ----------------
C = 8                 # cores
NPAD = 1408           # padded nodes per core (= 11 * 128)
NT_OWN = NPAD // 128  # 11 node tiles per core
NTOT = C * NPAD       # 11264 global padded node slots
ET = 168              # edge tiles per core (FFD-packed; assert fits)
ES = ET * 128         # 21504 edge slots per core
CH = 8                # edge tiles per gather chunk
NCHUNK = ET // CH     # 21
SEG = 32              # segment slots per tile (31 real + 1 trash)
ACCN = ET * SEG       # 5376 accumulation rows
ACC_CONST_MID = ACCN          # const row for zero-degree nodes (mid)
ACC_CONST_OUT = ACCN + 1      # const row (out layer)
ACC_ROWS = ACCN + 64
WB = 384              # B-table row width, layers 0-2
WB3 = 128             # B-table row width, layer 3
WA = 260              # accum row content width, mid (256 msg + 4 ex)
WA3 = 65              # accum row content width, out (64 msg + 1 ex)

_PROGRAM_CACHE = {}


def _wrap16(u):
    """int index vector [n] -> dma_gather idx layout [128, n//16] int16."""
    n = len(u)
    assert n % 16 == 0
    a = np.asarray(u).reshape(n // 16, 16).T
    return np.ascontiguousarray(np.tile(a, (8, 1)).astype(np.int16))


def _make_wn_aug(Wn, a):
    """Wn [fin,H,F], a [3,H,F] -> augmented node weight [fin, WB or WB3]."""
    fin, Hh, Ff = Wn.shape
    w = WB if Hh == 4 else WB3
    Wa = np.zeros((fin, w), np.float32)
    if Hh == 4:
        Wa[:, 0:256] = Wn.reshape(fin, 256)
        Wa[:, 256:320] = Wn.mean(axis=1)
        Wa[:, 320:324] = np.einsum("ihf,hf->ih", Wn, a[0])
    else:
        Wa[:, 0:64] = Wn[:, 0, :]
        Wa[:, 64] = np.einsum("if,f->i", Wn[:, 0, :], a[0, 0])
    return Wa


def _make_we_aug(We, a):
    fin, Hh, Ff = We.shape
    w = WB if Hh == 4 else WB3
    Wa = np.zeros((fin, w), np.float32)
    if Hh == 4:
        Wa[:, 0:256] = We.reshape(fin, 256)
        Wa[:, 256:320] = We.mean(axis=1)
        Wa[:, 320:324] = np.einsum("ihf,hf->ih", We, a[2])
    else:
        Wa[:, 0:64] = We[:, 0, :]
        Wa[:, 64] = np.einsum("if,f->i", We[:, 0, :], a[2, 0])
    return Wa


def _make_wdst(Wn, a):
    """dst-side row projector: x_d -> [mean_h_d(0:64) | s_dst(64:64+H) | 0]."""
    fin, Hh, Ff = Wn.shape
    Wa = np.zeros((fin, 128), np.float32)
    if Hh == 4:
        Wa[:, 0:64] = Wn.mean(axis=1)
        Wa[:, 64:68] = np.einsum("ihf,hf->ih", Wn, a[1])
    else:
        # out layer: edge_out unused, only the score is needed
        Wa[:, 64] = np.einsum("if,f->i", Wn[:, 0, :], a[1, 0])
    return Wa


def _f16(x):
    return np.ascontiguousarray(np.asarray(x, np.float32).astype(np.float16))


def _pack_core(dst_local_sorted):
    """FFD-pack whole dst-segments into tiles of <=128 edges, <=31 segments.

    dst_local_sorted: dst node (orig id) per edge, sorted ascending.
    Returns (tile_of_seg, segidx_of_seg, uniq, start, cnts, tile_edges)
    where seg i is the i-th unique dst in ascending-dst order.
    """
    uniq, start, cnts = np.unique(dst_local_sorted, return_index=True,
                                  return_counts=True)
    nseg = len(uniq)
    order = np.argsort(-cnts, kind="stable")   # big segments first
    tile_edges = []
    tile_nseg = []
    tile_of_seg = np.empty(nseg, np.int64)
    segidx_of_seg = np.empty(nseg, np.int64)
    for si in order:
        cnt = cnts[si]
        placed = False
        for t in range(len(tile_edges)):
            if tile_edges[t] + cnt <= 128 and tile_nseg[t] < SEG - 1:
                tile_of_seg[si] = t
                segidx_of_seg[si] = tile_nseg[t]
                tile_edges[t] += cnt
                tile_nseg[t] += 1
                placed = True
                break
        if not placed:
            tile_of_seg[si] = len(tile_edges)
            segidx_of_seg[si] = 0
            tile_edges.append(int(cnt))
            tile_nseg.append(1)
    assert len(tile_edges) <= ET, len(tile_edges)
    return tile_of_seg, segidx_of_seg, uniq, start, cnts, tile_edges


def _preprocess(inputs):
    """Host-side: relabel nodes, pack edges, build per-core device inputs."""
    nf = np.asarray(inputs["node_features"], np.float32)
    ef = np.asarray(inputs["edge_features"], np.float32)
    ei = np.asarray(inputs["edge_index"], np.int64)
    bv = np.asarray(inputs["batch_vector"], np.int64)

    core_of_node = bv // (G // C)
    core_of_edge = core_of_node[ei[1]]
    cnt_graph = np.bincount(bv, minlength=G).astype(np.float32)

    # ---- per-core packing (uses orig ids), then global relabel
    packs = []
    new_id = np.empty(N_NODES, np.int64)
    core_node_lists = []
    for c in range(C):
        eids = np.nonzero(core_of_edge == c)[0]
        eo = eids[np.argsort(ei[1][eids], kind="stable")]
        dsts = ei[1][eo]
        tile_of_seg, segidx_of_seg, uniq, start, cnts, tile_edges = \
            _pack_core(dsts)
        nseg = len(uniq)

        ntile = len(tile_edges)
        seg_order = np.lexsort((segidx_of_seg, tile_of_seg))
        off_in_tile = np.zeros(nseg, np.int64)
        fill = np.zeros(ntile, np.int64)
        for si in seg_order:
            t = tile_of_seg[si]
            off_in_tile[si] = fill[t]
            fill[t] += cnts[si]

        slot_of_edge = np.empty(len(eo), np.int64)
        seg_of_edge = np.empty(len(eo), np.int64)
        for si in range(nseg):
            s0 = start[si]
            cnt = cnts[si]
            slot_of_edge[s0:s0 + cnt] = (tile_of_seg[si] * 128
                                         + off_in_tile[si] + np.arange(cnt))
            seg_of_edge[s0:s0 + cnt] = segidx_of_seg[si]

        # node order: nodes with segments by (tile, segidx), then the rest
        nodes_with_seg = uniq[seg_order]
        own = np.nonzero(core_of_node == c)[0]
        rest = own[~np.isin(own, nodes_with_seg)]
        ordered = np.concatenate([nodes_with_seg, rest])
        assert len(ordered) <= NPAD, len(ordered)
        new_id[ordered] = c * NPAD + np.arange(len(ordered))
        core_node_lists.append(ordered)
        packs.append(dict(eo=eo, slot_of_edge=slot_of_edge,
                          seg_of_edge=seg_of_edge, nseg=nseg,
                          tile_of_seg=tile_of_seg, segidx_of_seg=segidx_of_seg,
                          seg_order=seg_order))

    x0T = np.zeros((F_IN, NTOT), np.float32)
    x0T[:, new_id] = nf.T
    src_new = new_id[ei[0]]

    per_core = []
    for c in range(C):
        p = packs[c]
        eo, slot_of_edge, seg_of_edge = p["eo"], p["slot_of_edge"], p["seg_of_edge"]
        nseg = p["nseg"]

        slot_src = np.zeros(ES, np.int64)
        slot_src[slot_of_edge] = src_new[eo]
        seg_of_slot = np.full(ES, SEG - 1, np.int64)
        seg_of_slot[slot_of_edge] = seg_of_edge

        # one-hot segment matrices (static): soh [128, ET*SEG], sohT [SEG, ET*128]
        soh = np.zeros((128, ET * SEG), np.float32)
        sohT = np.zeros((SEG, ET * 128), np.float32)
        sl = np.arange(ES)
        soh[sl % 128, (sl // 128) * SEG + seg_of_slot] = 1.0
        sohT[seg_of_slot, sl] = 1.0

        # node k (k-th in core order) -> ACC row / DSTTAB slot
        accslot = np.full(NPAD, -1, np.int64)
        so = p["seg_order"]
        accslot[0:nseg] = p["tile_of_seg"][so] * SEG + p["segidx_of_seg"][so]
        pos_m = accslot.copy()
        pos_m[pos_m < 0] = ACC_CONST_MID
        pos_o = accslot.copy()
        pos_o[pos_o < 0] = ACC_CONST_OUT
        dstscat = accslot.copy()
        dstscat[dstscat < 0] = ACCN       # trash row, never read back

        e0T_c = np.zeros((E_IN, ES), np.float32)
        e0T_c[:, slot_of_edge] = ef[eo].T

        # readout one-hot (scaled by 1/count) for the 8 local graphs
        g1h = np.zeros((128, NT_OWN * 8), np.float32)
        ids = core_node_lists[c]
        if len(ids):
            j = np.arange(len(ids))
            gl = bv[ids] % 8
            vals = 1.0 / np.maximum(cnt_graph[bv[ids]], 1.0)
            g1h[j % 128, (j // 128) * 8 + gl] = vals

        per_core.append(dict(
            e0T=_f16(e0T_c),
            srcidx=_wrap16(slot_src),
            soh=_f16(soh),
            sohT=_f16(sohT),
            accidx_m=_wrap16(pos_m),
            accidx_o=_wrap16(pos_o),
            dstscat=_wrap16(dstscat),
            g1h=g1h,
            x0own=_f16(x0T[:, c * NPAD:(c + 1) * NPAD]),
        ))

    # ---- weights
    Wn0 = np.asarray(inputs["Wn0"], np.float32)
    We0 = np.asarray(inputs["We0"], np.float32)
    a0 = np.asarray(inputs["a0"], np.float32)
    Wnm = np.asarray(inputs["Wn_mid"], np.float32)
    Wem = np.asarray(inputs["We_mid"], np.float32)
    am = np.asarray(inputs["a_mid"], np.float32)
    Wno = np.asarray(inputs["Wn_out"], np.float32)
    Weo = np.asarray(inputs["We_out"], np.float32)
    ao = np.asarray(inputs["a_out"], np.float32)

    shared = dict(
        x0T=_f16(x0T),
        accconst=np.zeros((2, 320), np.float32),
        wn0=_f16(_make_wn_aug(Wn0, a0)),
        we0=_f16(_make_we_aug(We0, a0)),
        wn1=_f16(_make_wn_aug(Wnm[0], am[0])),
        we1=_f16(_make_we_aug(Wem[0], am[0])),
        wn2=_f16(_make_wn_aug(Wnm[1], am[1])),
        we2=_f16(_make_we_aug(Wem[1], am[1])),
        wn3=_f16(_make_wn_aug(Wno, ao)),
        we3=_f16(_make_we_aug(Weo, ao)),
        wd0=_f16(_make_wdst(Wn0, a0)),
        wd1=_f16(_make_wdst(Wnm[0], am[0])),
        wd2=_f16(_make_wdst(Wnm[1], am[1])),
        wd3=_f16(_make_wdst(Wno, ao)),
    )
    shared["accconst"][0, 256:260] = 1.0   # mid const row: num=0, den=1
    shared["accconst"][1, 64] = 1.0        # out const row

    in_maps = []
    for c in range(C):
        m = dict(shared)
        m.update(per_core[c])
        in_maps.append({k: np.ascontiguousarray(v) for k, v in m.items()})
    return in_maps


def _build_program():
    from concourse import bacc, mybir, tile
    from concourse.masks import make_identity

    f32 = mybir.dt.float32
    f16 = mybir.dt.float16
    i16 = mybir.dt.int16
    AOP = mybir.AluOpType
    AF = mybir.ActivationFunctionType

    nc = bacc.Bacc("TRN2", target_bir_lowering=False, debug=False, num_devices=C)

    din = {}
    def dt(name, shape, dtype=f32, kind="ExternalInput"):
        din[name] = nc.dram_tensor(name, shape, dtype, kind=kind)
        return din[name]

    dt("x0T", [F_IN, NTOT], f16)
    dt("x0own", [F_IN, NPAD], f16)
    dt("e0T", [E_IN, ES], f16)
    dt("accconst", [2, 320])
    dt("srcidx", [128, ES // 16], i16)
    dt("soh", [128, ET * SEG], f16)
    dt("sohT", [SEG, ET * 128], f16)
    dt("accidx_m", [128, NPAD // 16], i16)
    dt("accidx_o", [128, NPAD // 16], i16)
    dt("dstscat", [128, NPAD // 16], i16)
    dt("g1h", [128, NT_OWN * 8])
    dt("wn0", [F_IN, WB], f16)
    dt("we0", [E_IN, WB], f16)
    for k in (1, 2):
        dt(f"wn{k}", [F, WB], f16)
        dt(f"we{k}", [F, WB], f16)
    dt("wn3", [F, WB3], f16)
    dt("we3", [F, WB3], f16)
    dt("wd0", [F_IN, 128], f16)
    for k in (1, 2, 3):
        dt(f"wd{k}", [F, 128], f16)
    out_t = dt("out", [8, OUT], kind="ExternalOutput")

    with tile.TileContext(nc) as tc:
        with tc.tile_pool(name="persist", bufs=1) as pp, \
             tc.tile_pool(name="work", bufs=2) as wp, \
             tc.tile_pool(name="work3", bufs=3) as wp3, \
             tc.tile_pool(name="pmain", bufs=3, space="PSUM") as pmain, \
             tc.tile_pool(name="pseg", bufs=2, space="PSUM") as pseg, \
             tc.tile_pool(name="ptr", bufs=2, space="PSUM") as ptr, \
             tc.tile_pool(name="pg", bufs=1, space="PSUM") as pgp, \
             tc.tile_pool(name="dram", bufs=1, space="DRAM") as dp:

            # ---- persistent SBUF
            def load_persist(name, shape, dtype=f32):
                t = pp.tile(shape, dtype, tag=name)
                nc.sync.dma_start(out=t[:], in_=din[name].ap())
                return t

            srcidx = load_persist("srcidx", [128, ES // 16], i16)
            soh = load_persist("soh", [128, ET * SEG], f16)
            sohT = load_persist("sohT", [SEG, ET * 128], f16)
            accidx_m = load_persist("accidx_m", [128, NPAD // 16], i16)
            accidx_o = load_persist("accidx_o", [128, NPAD // 16], i16)
            dstscat = load_persist("dstscat", [128, NPAD // 16], i16)
            g1h = load_persist("g1h", [128, NT_OWN * 8])
            x0own = load_persist("x0own", [F_IN, NPAD], f16)
            wn = [load_persist("wn0", [F_IN, WB], f16),
                  load_persist("wn1", [F, WB], f16),
                  load_persist("wn2", [F, WB], f16),
                  load_persist("wn3", [F, WB3], f16)]
            we = [load_persist("we0", [E_IN, WB], f16),
                  load_persist("we1", [F, WB], f16),
                  load_persist("we2", [F, WB], f16),
                  load_persist("we3", [F, WB3], f16)]
            wd = [load_persist("wd0", [F_IN, 128], f16),
                  load_persist("wd1", [F, 128], f16),
                  load_persist("wd2", [F, 128], f16),
                  load_persist("wd3", [F, 128], f16)]
            ident = pp.tile([128, 128], f16, tag="ident")
            make_identity(nc, ident[:])

            # ---- DRAM scratch
            B = [dp.tile([NTOT, WB], f16, tag="B0", name="B0"),
                 dp.tile([NTOT, WB], f16, tag="B1", name="B1"),
                 dp.tile([NTOT, WB], f16, tag="B2", name="B2"),
                 dp.tile([NTOT, WB3], f16, tag="B3", name="B3")]
            ACCD = dp.tile([ACC_ROWS, 320], f32, tag="ACC")
            EA = dp.tile([F, ES], f16, tag="EA")
            EB = dp.tile([F, ES], f16, tag="EB")
            AGIN = dp.tile([F, NPAD], f16, tag="AGIN")
            CCT = [dp.tile([C, F, NPAD], f16, tag=f"CC{k}", name=f"CC{k}",
                           addr_space="Shared") for k in range(3)]
            DTB = [dp.tile([ACCN + 64, 128], f16, tag=f"DT{k}", name=f"DT{k}")
                   for k in range(4)]

            # const rows + zero-init of DSTTABs
            nc.sync.dma_start(out=ACCD[ACC_CONST_MID:ACC_CONST_MID + 2, :],
                              in_=din["accconst"].ap())
            zeros = pp.tile([128, ACCN], f16, tag="zeros")
            nc.vector.memset(zeros[:], 0.0)
            for k in range(4):
                nc.sync.dma_start(
                    out=DTB[k][0:ACCN, :].rearrange("(a p) w -> p a w", p=128),
                    in_=zeros[:].rearrange("p (a w) -> p a w", w=128))

            # ------------------------------------------------------------------
            def build_dst0():
                """DSTTAB[0] rows from raw node features (own block)."""
                dstg = wp.tile([128, NT_OWN, 128], f16, tag="dstg", bufs=1,
                               name="dstg0")
                for nt in range(NT_OWN):
                    pd = pmain.tile([128, WB], f32, space="PSUM", tag="pmain")
                    nc.tensor.matmul(out=pd[:, 0:128],
                                     lhsT=x0own[:, nt * 128:(nt + 1) * 128],
                                     rhs=wd[0][:, :], start=True, stop=True)
                    if nt % 2 == 0:
                        nc.vector.tensor_copy(out=dstg[:, nt, :], in_=pd[:, 0:128])
                    else:
                        nc.scalar.copy(out=dstg[:, nt, :], in_=pd[:, 0:128])
                nc.gpsimd.dma_scatter_add(
                    out_ap=DTB[0][:, :], in_ap=dstg[:], idxs_ap=dstscat[:],
                    num_idxs=NPAD, num_idxs_reg=NPAD, elem_size=128)

            # ------------------------------------------------------------------
            def build_B(L):
                """Full-table node matmul: B[L] rows for all NTOT nodes."""
                WBL = 324 if L < 3 else WB3
                for blk in range(C):
                    if L == 0:
                        xb = wp.tile([F_IN, NPAD], f16, tag="xblk")
                        nc.sync.dma_start(out=xb[:], in_=din["x0T"].ap()[:, blk * NPAD:(blk + 1) * NPAD])
                        kn = F_IN
                    else:
                        xb = wp.tile([F, NPAD], f16, tag="xblk", name="xblkm")
                        nc.sync.dma_start(out=xb[:], in_=CCT[L - 1][blk, :, :])
                        kn = F
                    for j0 in range(0, NT_OWN, 4):
                        nb = min(4, NT_OWN - j0)
                        bs = wp.tile([128, 4, WB], f16, tag="bstage")
                        for dj in range(nb):
                            j = j0 + dj
                            pb = pmain.tile([128, WB], f32, space="PSUM", tag="pmain")
                            nc.tensor.matmul(out=pb[:, 0:WBL],
                                             lhsT=xb[:kn, j * 128:(j + 1) * 128],
                                             rhs=wn[L][:kn, 0:WBL],
                                             start=True, stop=True)
                            if dj % 2 == 0:
                                nc.vector.tensor_copy(out=bs[:, dj, 0:WBL], in_=pb[:, 0:WBL])
                            else:
                                nc.scalar.copy(out=bs[:, dj, 0:WBL], in_=pb[:, 0:WBL])
                        nt = blk * NT_OWN + j0
                        nc.sync.dma_start(
                            out=B[L][nt * 128:(nt + nb) * 128, 0:WBL].rearrange(
                                "(b p) w -> p b w", p=128),
                            in_=bs[:, 0:nb, 0:WBL])

            # ------------------------------------------------------------------
            def edge_phase(L):
                mid = L < 3
                WBL = WB if mid else WB3
                WE = 324 if mid else 128      # cols actually used per B row
                Hk = H if mid else 1
                MS = 256 if mid else 64       # msg cols
                WAk = WA if mid else WA3      # acc row content
                SS = 320 if mid else 64       # col of summed scores
                DOFF = 256 if mid else 0      # psum col where dst-mm lands
                DW = 68 if mid else 128       # dst row cols used
                exscale = float(Hk)
                esrc = [None, EA, EB, EA][L]
                edst = [EA, EB, EA, None][L]

                for c in range(NCHUNK):
                    cs, ce = c * CH * 128, (c + 1) * CH * 128
                    gb = wp.tile([128, CH, WBL], f16, tag="gb", name="gb")
                    nc.gpsimd.dma_gather(
                        out_ap=gb[:], in_ap=B[L][:, :],
                        idxs_ap=srcidx[:, c * (CH * 8):(c + 1) * (CH * 8)],
                        num_idxs=CH * 128, num_idxs_reg=CH * 128, elem_size=WBL)
                    dr = wp.tile([SEG, CH, 128], f16, tag="dr", name="dr")
                    nc.sync.dma_start(
                        out=dr[:],
                        in_=DTB[L][c * CH * SEG:(c + 1) * CH * SEG, :].rearrange(
                            "(t s) w -> s t w", s=SEG))
                    if L == 0:
                        ech = wp.tile([E_IN, CH * 128], f16, tag="ech0")
                        nc.sync.dma_start(out=ech[:], in_=din["e0T"].ap()[:, cs:ce])
                        ke = E_IN
                    else:
                        ech = wp.tile([F, CH * 128], f16, tag="ech")
                        nc.sync.dma_start(out=ech[:], in_=esrc[0:F, cs:ce])
                        ke = F

                    tch = wp.tile([128, CH, WE], f16, tag="tch")
                    for tl in range(CH):
                        t = c * CH + tl
                        pm = pmain.tile([128, WB], f32, space="PSUM", tag="pmain")
                        nc.tensor.matmul(out=pm[:, 0:WE],
                                         lhsT=ech[:ke, tl * 128:(tl + 1) * 128],
                                         rhs=we[L][:ke, 0:WE], start=True,
                                         stop=False, skip_group_check=True)
                        nc.tensor.matmul(out=pm[:, DOFF:DOFF + DW],
                                         lhsT=sohT[:, t * 128:(t + 1) * 128],
                                         rhs=dr[:, tl, 0:DW], start=False,
                                         stop=True, skip_group_check=True)
                        nc.vector.tensor_tensor(out=tch[:, tl, :], in0=pm[:, 0:WE],
                                                in1=gb[:, tl, 0:WE], op=AOP.add)

                    # ---- batched ops over the whole chunk (all SBUF, compact out)
                    lk = wp3.tile([128, CH, H], f16, tag="lk")
                    nc.vector.tensor_scalar_mul(lk[:, :, 0:Hk], tch[:, :, SS:SS + Hk],
                                                NEG_SLOPE)
                    lm = wp3.tile([128, CH, H], f16, tag="lm")
                    nc.vector.tensor_tensor(out=lm[:, :, 0:Hk],
                                            in0=tch[:, :, SS:SS + Hk],
                                            in1=lk[:, :, 0:Hk], op=AOP.max)
                    ex = wp3.tile([128, CH, H], f16, tag="ex")
                    nc.scalar.activation(ex[:, :, 0:Hk], lm[:, :, 0:Hk], AF.Exp)
                    if mid:
                        denw = wp3.tile([128, CH, H], f16, tag="denw")
                        nc.vector.tensor_scalar_mul(denw[:], ex[:], exscale)
                    else:
                        denw = ex
                    mwm = wp.tile([128, CH, 256], f16, tag="mwm")
                    nc.vector.tensor_tensor(
                        out=mwm[:, :, 0:MS].rearrange("p c (h f) -> p c h f", h=Hk),
                        in0=tch[:, :, 0:MS].rearrange("p c (h f) -> p c h f", h=Hk),
                        in1=ex[:, :, 0:Hk].unsqueeze(3).to_broadcast([128, CH, Hk, F]),
                        op=AOP.mult)
                    if mid:
                        eech = wp.tile([128, CH, F], f16, tag="eech")
                        nc.scalar.activation(eech[:], tch[:, :, 256:320], AF.Exp)
                        eem = wp.tile([128, CH, F], f16, tag="eem")
                        nc.vector.tensor_scalar(out=eem[:], in0=eech[:], scalar1=-1.0,
                                                scalar2=None, op0=AOP.add)
                        rrch = wp.tile([128, CH, F], f16, tag="rrch")
                        nc.vector.tensor_scalar(out=rrch[:], in0=tch[:, :, 256:320],
                                                scalar1=0.0, scalar2=None, op0=AOP.max)
                        ench = wp.tile([128, CH, F], f16, tag="ench")
                        nc.vector.tensor_tensor(out=ench[:], in0=eem[:], in1=rrch[:],
                                                op=AOP.min)
                        est = wp.tile([128, CH // 2, 128], f16, tag="est")

                    acst = wp.tile([64, 4, WA], f32, tag="acst")
                    for g in range(4):
                        ps = pseg.tile([64, WA], f32, space="PSUM", tag="pseg")
                        for q in range(2):
                            tl = g * 2 + q
                            t = c * CH + tl
                            nc.tensor.matmul(out=ps[q * SEG:(q + 1) * SEG, 0:MS],
                                             lhsT=soh[:, t * SEG:(t + 1) * SEG],
                                             rhs=mwm[:, tl, 0:MS], start=True,
                                             stop=False, skip_group_check=True)
                            nc.tensor.matmul(out=ps[q * SEG:(q + 1) * SEG, MS:MS + Hk],
                                             lhsT=soh[:, t * SEG:(t + 1) * SEG],
                                             rhs=denw[:, tl, 0:Hk], start=False,
                                             stop=True, skip_group_check=True)
                        nc.scalar.copy(out=acst[:, g, 0:WAk], in_=ps[:, 0:WAk])
                    if mid:
                        for pr2 in range(CH // 2):
                            pt = ptr.tile([128, 128], f16, space="PSUM", tag="ptr")
                            nc.tensor.transpose(out=pt[0:F, :], in_=ench[:, 2 * pr2, :],
                                                identity=ident[:])
                            nc.tensor.transpose(out=pt[F:128, :], in_=ench[:, 2 * pr2 + 1, :],
                                                identity=ident[:])
                            if pr2 % 2 == 0:
                                nc.vector.tensor_copy(out=est[:, pr2, :], in_=pt[:])
                            else:
                                nc.scalar.copy(out=est[:, pr2, :], in_=pt[:])
                    nc.sync.dma_start(
                        out=ACCD[c * CH * SEG:(c + 1) * CH * SEG, 0:WAk].rearrange(
                            "(g p) w -> p g w", p=64),
                        in_=acst[:, :, 0:WAk])
                    if mid:
                        nc.sync.dma_start(
                            out=edst[0:F, cs:ce].rearrange(
                                "f (i j p) -> f j i p", j=2, p=128)[:, 0, :, :],
                            in_=est[0:F, :, :])
                        nc.sync.dma_start(
                            out=edst[0:F, cs:ce].rearrange(
                                "f (i j p) -> f j i p", j=2, p=128)[:, 1, :, :],
                            in_=est[F:128, :, :])

            # ------------------------------------------------------------------
            def node_phase(L):
                if L < 3:
                    gn = wp.tile([128, NT_OWN, 320], f32, tag="gn", bufs=1)
                    nc.gpsimd.dma_gather(
                        out_ap=gn[:, 0:8, :], in_ap=ACCD[:, :], idxs_ap=accidx_m[:, 0:64],
                        num_idxs=1024, num_idxs_reg=1024, elem_size=320)
                    nc.gpsimd.dma_gather(
                        out_ap=gn[:, 8:NT_OWN, :], in_ap=ACCD[:, :], idxs_ap=accidx_m[:, 64:88],
                        num_idxs=NPAD - 1024, num_idxs_reg=NPAD - 1024, elem_size=320)
                    xstg = wp.tile([F, NT_OWN, 128], f16, tag="xstg", bufs=1)
                    dstg = wp.tile([128, NT_OWN, 128], f16, tag="dstg", bufs=1)
                    for nt in range(NT_OWN):
                        rec = wp3.tile([128, H], f32, tag="rec")
                        nc.vector.reciprocal(out=rec[:], in_=gn[:, nt, 256:260])
                        pr = wp3.tile([128, 256], f32, tag="pr")
                        nc.vector.tensor_tensor(
                            out=pr[:].rearrange("p (h f) -> p h f", h=H),
                            in0=gn[:, nt, 0:256].rearrange("p (h f) -> p h f", h=H),
                            in1=rec[:].unsqueeze(2).to_broadcast([128, H, F]),
                            op=AOP.mult)
                        xo = wp3.tile([128, F], f32, tag="xo")
                        nc.vector.tensor_reduce(
                            out=xo[:], in_=pr[:].rearrange("p (h f) -> p f h", h=H),
                            axis=mybir.AxisListType.X, op=AOP.add)
                        # ELU(x) = min(exp(x)-1, max(x, 0))
                        xe = wp3.tile([128, F], f16, tag="ee")
                        nc.scalar.activation(xe[:], xo[:], AF.Exp)
                        xem = wp3.tile([128, F], f16, tag="eem1")
                        nc.vector.tensor_scalar(out=xem[:], in0=xe[:], scalar1=-1.0,
                                                scalar2=None, op0=AOP.add)
                        xr = wp3.tile([128, F], f16, tag="rr")
                        nc.vector.tensor_scalar(out=xr[:], in0=xo[:], scalar1=0.0,
                                                scalar2=None, op0=AOP.max)
                        xs = wp3.tile([128, F], f16, tag="en")
                        nc.vector.tensor_tensor(out=xs[:], in0=xem[:], in1=xr[:], op=AOP.min)
                        pt = ptr.tile([F, 128], f16, space="PSUM", tag="ptr")
                        nc.tensor.transpose(out=pt[:], in_=xs[:], identity=ident[:])
                        if nt % 2 == 0:
                            nc.vector.tensor_copy(out=xstg[:, nt, :], in_=pt[:])
                        else:
                            nc.scalar.copy(out=xstg[:, nt, :], in_=pt[:])
                        # next layer's dst-table rows (node-major)
                        pd = pmain.tile([128, WB], f32, space="PSUM", tag="pmain")
                        nc.tensor.matmul(out=pd[:, 0:128],
                                         lhsT=xstg[:, nt, :], rhs=wd[L + 1][:, :],
                                         start=True, stop=True)
                        if nt % 2 == 0:
                            nc.scalar.copy(out=dstg[:, nt, :], in_=pd[:, 0:128])
                        else:
                            nc.vector.tensor_copy(out=dstg[:, nt, :], in_=pd[:, 0:128])
                    nc.sync.dma_start(out=AGIN[:, :].rearrange("f (t p) -> f t p", p=128),
                                      in_=xstg[:])
                    nc.gpsimd.dma_scatter_add(
                        out_ap=DTB[L + 1][:, :], in_ap=dstg[:], idxs_ap=dstscat[:],
                        num_idxs=NPAD, num_idxs_reg=NPAD, elem_size=128)
                    nc.gpsimd.collective_compute(
                        "AllGather", AOP.bypass,
                        replica_groups=[list(range(C))],
                        ins=[AGIN[:]], outs=[CCT[L][:]])
                    build_B(L + 1)
                else:
                    gn = wp.tile([128, NT_OWN, 128], f32, tag="gn", name="gn3", bufs=1)
                    nc.gpsimd.dma_gather(
                        out_ap=gn[:, 0:8, :], in_ap=ACCD[:, 0:128], idxs_ap=accidx_o[:, 0:64],
                        num_idxs=1024, num_idxs_reg=1024, elem_size=128, elem_step=320)
                    nc.gpsimd.dma_gather(
                        out_ap=gn[:, 8:NT_OWN, :], in_ap=ACCD[:, 0:128], idxs_ap=accidx_o[:, 64:88],
                        num_idxs=NPAD - 1024, num_idxs_reg=NPAD - 1024, elem_size=128, elem_step=320)
                    pg = pgp.tile([8, OUT], f32, space="PSUM", tag="pg")
                    for nt in range(NT_OWN):
                        rec = wp3.tile([128, H], f32, tag="rec")
                        nc.vector.reciprocal(out=rec[:, 0:1], in_=gn[:, nt, 64:65])
                        nod = wp3.tile([128, 256], f32, tag="pr", name="nod")
                        nc.vector.tensor_tensor(out=nod[:, 0:OUT], in0=gn[:, nt, 0:OUT],
                                                in1=rec[:, 0:1].to_broadcast([128, OUT]),
                                                op=AOP.mult)
                        nc.tensor.matmul(out=pg[:], lhsT=g1h[:, nt * 8:(nt + 1) * 8],
                                         rhs=nod[:, 0:OUT], start=(nt == 0),
                                         stop=(nt == NT_OWN - 1), skip_group_check=True)
                    og = wp3.tile([8, OUT], f32, tag="og")
                    nc.vector.tensor_copy(out=og[:], in_=pg[:])
                    nc.sync.dma_start(out=out_t.ap(), in_=og[:])

            # ------------------------------------------------------------------
            stage = os.environ.get("KERNEL_STAGE", "full")
            og0 = wp3.tile([8, OUT], f32, tag="og", name="og0")
            nc.vector.memset(og0[:], 0.0)
            nc.sync.dma_start(out=out_t.ap(), in_=og0[:])
            if stage == "full":
                build_dst0()
                build_B(0)
                for L in range(4):
                    edge_phase(L)
                    node_phase(L)
            else:
                n = int(stage)  # 1=B0+dst0, 2=+edge0, 3=+node0, 4=+edge1, ...
                step = 0
                build_dst0()
                build_B(0)
                step += 1
                for L in range(4):
                    if step >= n:
                        break
                    edge_phase(L)
                    step += 1
                    if step >= n:
                        break
                    node_phase(L)
                    step += 1

    nc.compile()
    return nc


def _get_program():
    if "nc" not in _PROGRAM_CACHE:
        _PROGRAM_CACHE["nc"] = _build_program()
    return _PROGRAM_CACHE["nc"]


def kernel(**inputs):
    from concourse.bass_utils import run_bass_kernel_spmd

    nc = _get_program()
    in_maps = _preprocess(inputs)
    trace = bool(int(os.environ.get("KERNEL_TRACE", "0")))
    res = run_bass_kernel_spmd(nc, in_maps, core_ids=list(range(C)), trace=trace)
    _PROGRAM_CACHE["last_result"] = res
    out = np.concatenate([np.asarray(res.results[c]["out"]) for c in range(C)], axis=0)
    return out.astype(np.float32)


# revision 13
# speedup vs baseline: 1.5391x; 1.0483x over previous
"""GAT-with-edge-features GNN on 8 Trainium2 NeuronCores.

Strategy (self-contained; shapes hardcoded for the fixed problem size):
  - Relabel nodes so each core owns a contiguous block of NPAD node slots,
    grouped graph-wise by batch_vector (core = graph // 8).
  - Partition edges by the owning core of their dst node; within a core,
    sort edges by dst and FFD-bin-pack whole dst-segments into 128-edge
    tiles (<= 31 segments per tile) so every segment lives in one tile.
  - Every core redundantly computes the full node-level table B (fp16) so
    per-edge h[src] is a row gather from HBM (gpsimd dma_gather).  B rows
    interleave a constant column into each head block ([h_h | 4.0] x H)
    so the ex-weighted message AND the softmax denominator come out of a
    single elementwise multiply + a single one-hot segment matmul.  The
    dst-side values (mean_h_d, s_dst) are kept in a slot-ordered table
    DSTTAB[L] built by a 1408-row dma_scatter_add during the previous
    node phase, then broadcast to edges with a per-tile one-hot matmul
    fused into the same PSUM as the edge-feature projection.  Softmax is
    denominators-folded: node_out = (sum_e ex*(hs+ep)) / (sum_e ex).
    Between layers, per-core node states are AllGathered (fp16, Shared).
"""
import sys
import os

for _p in ("/opt/trn_rl_repo", "/root/.axon_site/_ro/trn_rl_repo"):
    if os.path.isdir(_p) and _p not in sys.path:
        sys.path.insert(0, _p)

import numpy as np

# ---------------- problem constants (hardcoded from spec) ----------------
N_NODES = 10000
N_EDGES = 160000
F_IN = 128
E_IN = 32
F = 64
H = 4
OUT = 64
L_MID = 2
G = 64
NEG_SLOPE = 0.2

# ---------------- sharding constants ----------------
C = 8                 # cores
NPAD = 1408           # padded nodes per core (= 11 * 128)
NT_OWN = NPAD // 128  # 11 node tiles per core
NTOT = C * NPAD       # 11264 global padded node slots
ET = 168              # edge tiles per core (FFD-packed; assert fits)
ES = ET * 128         # 21504 edge slots per core
CH = 8                # edge tiles per gather chunk
NCHUNK = ET // CH     # 21
SEG = 32              # segment slots per tile (31 real + 1 trash)
ACCN = ET * SEG       # 5376 accumulation rows
ACC_CONST_MID = ACCN          # const row for zero-degree nodes (mid)
ACC_CONST_OUT = ACCN + 1      # const row (out layer)
ACC_ROWS = ACCN + 64
WB = 384              # B-table row stride, layers 0-2
WB3 = 128             # B-table row stride, layer 3
WE_MID = 328          # used B-row cols, layers 0-2: 4*65 msg+den | 64 mean | 4 s
WE_OUT = 66           # used B-row cols, layer 3: 64 msg | 1 den | 1 s
WA = 260              # accum row content width, mid (4 x [64 msg | den])
WA3 = 65              # accum row content width, out (64 msg | den)

_PROGRAM_CACHE = {}


def _wrap16(u):
    """int index vector [n] -> dma_gather idx layout [128, n//16] int16."""
    n = len(u)
    assert n % 16 == 0
    a = np.asarray(u).reshape(n // 16, 16).T
    return np.ascontiguousarray(np.tile(a, (8, 1)).astype(np.int16))


def _interleave_heads(Wh):
    """Wh [fin, H, F] -> [fin, H*(F+1)] with a zero col after each head."""
    fin = Wh.shape[0]
    out = np.zeros((fin, H * (F + 1)), np.float32)
    for h in range(H):
        out[:, h * (F + 1):h * (F + 1) + F] = Wh[:, h, :]
    return out


def _make_wn_aug(Wn, a, ones_row, den_val):
    """Wn [fin,H,F], a [3,H,F] -> node weight [fin(+1), WE_MID or WE_OUT].

    Row layout (mid): [ (h_h | den) x4 | mean(64) | s_src(4) ].
    ones_row: append a row driven by the constant-1 input row; it carries
    den_val into each den column.
    """
    fin, Hh, Ff = Wn.shape
    rows = fin + (1 if ones_row else 0)
    if Hh == 4:
        Wa = np.zeros((rows, WE_MID), np.float32)
        Wa[:fin, 0:260] = _interleave_heads(Wn)
        Wa[:fin, 260:324] = Wn.mean(axis=1)
        Wa[:fin, 324:328] = np.einsum("ihf,hf->ih", Wn, a[0])
        if ones_row:
            Wa[fin, 64:260:65] = den_val
    else:
        Wa = np.zeros((rows, WE_OUT), np.float32)
        Wa[:fin, 0:64] = Wn[:, 0, :]
        Wa[:fin, 65] = np.einsum("if,f->i", Wn[:, 0, :], a[0, 0])
        if ones_row:
            Wa[fin, 64] = den_val
    return Wa


def _make_we_aug(We, a, ones_row, den_val):
    fin, Hh, Ff = We.shape
    rows = fin + (1 if ones_row else 0)
    if Hh == 4:
        Wa = np.zeros((rows, WE_MID), np.float32)
        Wa[:fin, 0:260] = _interleave_heads(We)
        Wa[:fin, 260:324] = We.mean(axis=1)
        Wa[:fin, 324:328] = np.einsum("ihf,hf->ih", We, a[2])
        if ones_row:
            Wa[fin, 64:260:65] = den_val
    else:
        Wa = np.zeros((rows, WE_OUT), np.float32)
        Wa[:fin, 0:64] = We[:, 0, :]
        Wa[:fin, 65] = np.einsum("if,f->i", We[:, 0, :], a[2, 0])
        if ones_row:
            Wa[fin, 64] = den_val
    return Wa


def _make_wdst(Wn, a, pad_rows):
    """dst-side row projector: x_d -> [mean_h_d(0:64) | s_dst | 0]."""
    fin, Hh, Ff = Wn.shape
    Wa = np.zeros((fin + pad_rows, 128), np.float32)
    if Hh == 4:
        Wa[:fin, 0:64] = Wn.mean(axis=1)
        Wa[:fin, 64:68] = np.einsum("ihf,hf->ih", Wn, a[1])
    else:
        # out layer: edge_out unused; score lands at tch col 65
        Wa[:fin, 65] = np.einsum("if,f->i", Wn[:, 0, :], a[1, 0])
    return Wa


def _f16(x):
    return np.ascontiguousarray(np.asarray(x, np.float32).astype(np.float16))


def _pack_core(dst_local_sorted):
    """FFD-pack whole dst-segments into tiles of <=128 edges, <=31 segments."""
    uniq, start, cnts = np.unique(dst_local_sorted, return_index=True,
                                  return_counts=True)
    nseg = len(uniq)
    order = np.argsort(-cnts, kind="stable")   # big segments first
    tile_edges = []
    tile_nseg = []
    tile_of_seg = np.empty(nseg, np.int64)
    segidx_of_seg = np.empty(nseg, np.int64)
    for si in order:
        cnt = cnts[si]
        placed = False
        for t in range(len(tile_edges)):
            if tile_edges[t] + cnt <= 128 and tile_nseg[t] < SEG - 1:
                tile_of_seg[si] = t
                segidx_of_seg[si] = tile_nseg[t]
                tile_edges[t] += cnt
                tile_nseg[t] += 1
                placed = True
                break
        if not placed:
            tile_of_seg[si] = len(tile_edges)
            segidx_of_seg[si] = 0
            tile_edges.append(int(cnt))
            tile_nseg.append(1)
    assert len(tile_edges) <= ET, len(tile_edges)
    return tile_of_seg, segidx_of_seg, uniq, start, cnts, tile_edges


def _preprocess(inputs):
    """Host-side: relabel nodes, pack edges, build per-core device inputs."""
    nf = np.asarray(inputs["node_features"], np.float32)
    ef = np.asarray(inputs["edge_features"], np.float32)
    ei = np.asarray(inputs["edge_index"], np.int64)
    bv = np.asarray(inputs["batch_vector"], np.int64)

    core_of_node = bv // (G // C)
    core_of_edge = core_of_node[ei[1]]
    cnt_graph = np.bincount(bv, minlength=G).astype(np.float32)

    # ---- per-core packing (uses orig ids), then global relabel
    packs = []
    new_id = np.empty(N_NODES, np.int64)
    core_node_lists = []
    for c in range(C):
        eids = np.nonzero(core_of_edge == c)[0]
        eo = eids[np.argsort(ei[1][eids], kind="stable")]
        dsts = ei[1][eo]
        tile_of_seg, segidx_of_seg, uniq, start, cnts, tile_edges = \
            _pack_core(dsts)
        nseg = len(uniq)

        ntile = len(tile_edges)
        seg_order = np.lexsort((segidx_of_seg, tile_of_seg))
        off_in_tile = np.zeros(nseg, np.int64)
        fill = np.zeros(ntile, np.int64)
        for si in seg_order:
            t = tile_of_seg[si]
            off_in_tile[si] = fill[t]
            fill[t] += cnts[si]

        slot_of_edge = np.empty(len(eo), np.int64)
        seg_of_edge = np.empty(len(eo), np.int64)
        for si in range(nseg):
            s0 = start[si]
            cnt = cnts[si]
            slot_of_edge[s0:s0 + cnt] = (tile_of_seg[si] * 128
                                         + off_in_tile[si] + np.arange(cnt))
            seg_of_edge[s0:s0 + cnt] = segidx_of_seg[si]

        nodes_with_seg = uniq[seg_order]
        own = np.nonzero(core_of_node == c)[0]
        rest = own[~np.isin(own, nodes_with_seg)]
        ordered = np.concatenate([nodes_with_seg, rest])
        assert len(ordered) <= NPAD, len(ordered)
        new_id[ordered] = c * NPAD + np.arange(len(ordered))
        core_node_lists.append(ordered)
        packs.append(dict(eo=eo, slot_of_edge=slot_of_edge,
                          seg_of_edge=seg_of_edge, nseg=nseg,
                          tile_of_seg=tile_of_seg, segidx_of_seg=segidx_of_seg,
                          seg_order=seg_order))

    x0T = np.zeros((F_IN, NTOT), np.float32)
    x0T[:, new_id] = nf.T
    src_new = new_id[ei[0]]

    per_core = []
    for c in range(C):
        p = packs[c]
        eo, slot_of_edge, seg_of_edge = p["eo"], p["slot_of_edge"], p["seg_of_edge"]
        nseg = p["nseg"]

        slot_src = np.zeros(ES, np.int64)
        slot_src[slot_of_edge] = src_new[eo]
        seg_of_slot = np.full(ES, SEG - 1, np.int64)
        seg_of_slot[slot_of_edge] = seg_of_edge

        soh = np.zeros((128, ET * SEG), np.float32)
        sohT = np.zeros((SEG, ET * 128), np.float32)
        sl = np.arange(ES)
        soh[sl % 128, (sl // 128) * SEG + seg_of_slot] = 1.0
        sohT[seg_of_slot, sl] = 1.0

        accslot = np.full(NPAD, -1, np.int64)
        so = p["seg_order"]
        accslot[0:nseg] = p["tile_of_seg"][so] * SEG + p["segidx_of_seg"][so]
        pos_m = accslot.copy()
        pos_m[pos_m < 0] = ACC_CONST_MID
        pos_o = accslot.copy()
        pos_o[pos_o < 0] = ACC_CONST_OUT
        dstscat = accslot.copy()
        dstscat[dstscat < 0] = ACCN       # trash row, never read back

        # edge features with a ones row (drives the den columns at L0)
        e0T_c = np.zeros((E_IN + 1, ES), np.float32)
        e0T_c[:E_IN, slot_of_edge] = ef[eo].T
        e0T_c[E_IN, :] = 1.0

        g1h = np.zeros((128, NT_OWN * 8), np.float32)
        ids = core_node_lists[c]
        if len(ids):
            j = np.arange(len(ids))
            gl = bv[ids] % 8
            vals = 1.0 / np.maximum(cnt_graph[bv[ids]], 1.0)
            g1h[j % 128, (j // 128) * 8 + gl] = vals

        per_core.append(dict(
            e0T=_f16(e0T_c),
            srcidx=_wrap16(slot_src),
            soh=_f16(soh),
            sohT=_f16(sohT),
            accidx_m=_wrap16(pos_m),
            accidx_o=_wrap16(pos_o),
            dstscat=_wrap16(dstscat),
            g1h=g1h,
            x0own=_f16(x0T[:, c * NPAD:(c + 1) * NPAD]),
        ))

    # ---- weights
    Wn0 = np.asarray(inputs["Wn0"], np.float32)
    We0 = np.asarray(inputs["We0"], np.float32)
    a0 = np.asarray(inputs["a0"], np.float32)
    Wnm = np.asarray(inputs["Wn_mid"], np.float32)
    Wem = np.asarray(inputs["We_mid"], np.float32)
    am = np.asarray(inputs["a_mid"], np.float32)
    Wno = np.asarray(inputs["Wn_out"], np.float32)
    Weo = np.asarray(inputs["We_out"], np.float32)
    ao = np.asarray(inputs["a_out"], np.float32)

    shared = dict(
        x0T=_f16(x0T),
        e_ones=_f16(np.ones((1, ES), np.float32)),
        accconst=np.zeros((2, 320), np.float32),
        # L0: den cols come from the e0T ones row (B0 has no ones input)
        wn0=_f16(_make_wn_aug(Wn0, a0, False, 0.0)),
        we0=_f16(_make_we_aug(We0, a0, True, float(H))),
        # mid: den cols come from the x-table ones row via wn
        wn1=_f16(_make_wn_aug(Wnm[0], am[0], True, float(H))),
        we1=_f16(_make_we_aug(Wem[0], am[0], False, 0.0)),
        wn2=_f16(_make_wn_aug(Wnm[1], am[1], True, float(H))),
        we2=_f16(_make_we_aug(Wem[1], am[1], False, 0.0)),
        wn3=_f16(_make_wn_aug(Wno, ao, True, 1.0)),
        we3=_f16(_make_we_aug(Weo, ao, False, 0.0)),
        wd0=_f16(_make_wdst(Wn0, a0, 0)),
        wd1=_f16(_make_wdst(Wnm[0], am[0], 1)),
        wd2=_f16(_make_wdst(Wnm[1], am[1], 1)),
        wd3=_f16(_make_wdst(Wno, ao, 1)),
    )
    shared["accconst"][0, 64:260:65] = 1.0   # mid const row: num=0, den=1
    shared["accconst"][1, 64] = 1.0          # out const row

    in_maps = []
    for c in range(C):
        m = dict(shared)
        m.update(per_core[c])
        in_maps.append({k: np.ascontiguousarray(v) for k, v in m.items()})
    return in_maps


def _build_program():
    from concourse import bacc, mybir, tile
    from concourse.masks import make_identity

    f32 = mybir.dt.float32
    f16 = mybir.dt.float16
    i16 = mybir.dt.int16
    AOP = mybir.AluOpType
    AF = mybir.ActivationFunctionType

    nc = bacc.Bacc("TRN2", target_bir_lowering=False, debug=False, num_devices=C)

    din = {}
    def dt(name, shape, dtype=f32, kind="ExternalInput"):
        din[name] = nc.dram_tensor(name, shape, dtype, kind=kind)
        return din[name]

    dt("x0T", [F_IN, NTOT], f16)
    dt("x0own", [F_IN, NPAD], f16)
    dt("e0T", [E_IN + 1, ES], f16)
    dt("e_ones", [1, ES], f16)
    dt("accconst", [2, 320])
    dt("srcidx", [128, ES // 16], i16)
    dt("soh", [128, ET * SEG], f16)
    dt("sohT", [SEG, ET * 128], f16)
    dt("accidx_m", [128, NPAD // 16], i16)
    dt("accidx_o", [128, NPAD // 16], i16)
    dt("dstscat", [128, NPAD // 16], i16)
    dt("g1h", [128, NT_OWN * 8])
    dt("wn0", [F_IN, WE_MID], f16)
    dt("we0", [E_IN + 1, WE_MID], f16)
    for k in (1, 2):
        dt(f"wn{k}", [F + 1, WE_MID], f16)
        dt(f"we{k}", [F, WE_MID], f16)
    dt("wn3", [F + 1, WE_OUT], f16)
    dt("we3", [F, WE_OUT], f16)
    dt("wd0", [F_IN, 128], f16)
    for k in (1, 2, 3):
        dt(f"wd{k}", [F + 1, 128], f16)
    out_t = dt("out", [8, OUT], kind="ExternalOutput")

    with tile.TileContext(nc) as tc:
        with tc.tile_pool(name="persist", bufs=1) as pp, \
             tc.tile_pool(name="work", bufs=2) as wp, \
             tc.tile_pool(name="work3", bufs=3) as wp3, \
             tc.tile_pool(name="pmain", bufs=3, space="PSUM") as pmain, \
             tc.tile_pool(name="pseg", bufs=2, space="PSUM") as pseg, \
             tc.tile_pool(name="ptr", bufs=2, space="PSUM") as ptr, \
             tc.tile_pool(name="pg", bufs=1, space="PSUM") as pgp, \
             tc.tile_pool(name="dram", bufs=1, space="DRAM") as dp:

            # ---- persistent SBUF
            def load_persist(name, shape, dtype=f32):
                t = pp.tile(shape, dtype, tag=name)
                nc.sync.dma_start(out=t[:], in_=din[name].ap())
                return t

            srcidx = load_persist("srcidx", [128, ES // 16], i16)
            soh = load_persist("soh", [128, ET * SEG], f16)
            sohT = load_persist("sohT", [SEG, ET * 128], f16)
            accidx_m = load_persist("accidx_m", [128, NPAD // 16], i16)
            accidx_o = load_persist("accidx_o", [128, NPAD // 16], i16)
            dstscat = load_persist("dstscat", [128, NPAD // 16], i16)
            g1h = load_persist("g1h", [128, NT_OWN * 8])
            x0own = load_persist("x0own", [F_IN, NPAD], f16)
            wn = [load_persist("wn0", [F_IN, WE_MID], f16),
                  load_persist("wn1", [F + 1, WE_MID], f16),
                  load_persist("wn2", [F + 1, WE_MID], f16),
                  load_persist("wn3", [F + 1, WE_OUT], f16)]
            we = [load_persist("we0", [E_IN + 1, WE_MID], f16),
                  load_persist("we1", [F, WE_MID], f16),
                  load_persist("we2", [F, WE_MID], f16),
                  load_persist("we3", [F, WE_OUT], f16)]
            wd = [load_persist("wd0", [F_IN, 128], f16),
                  load_persist("wd1", [F + 1, 128], f16),
                  load_persist("wd2", [F + 1, 128], f16),
                  load_persist("wd3", [F + 1, 128], f16)]
            ident = pp.tile([128, 128], f16, tag="ident")
            make_identity(nc, ident[:])

            # ---- DRAM scratch
            B = [dp.tile([NTOT, WB], f16, tag="B0", name="B0"),
                 dp.tile([NTOT, WB], f16, tag="B1", name="B1"),
                 dp.tile([NTOT, WB], f16, tag="B2", name="B2"),
                 dp.tile([NTOT, WB3], f16, tag="B3", name="B3")]
            ACCD = dp.tile([ACC_ROWS, 320], f32, tag="ACC")
            EA = dp.tile([F, ES], f16, tag="EA")
            EB = dp.tile([F, ES], f16, tag="EB")
            AGIN = dp.tile([F + 1, NPAD], f16, tag="AGIN")
            CCT = [dp.tile([C, F + 1, NPAD], f16, tag=f"CC{k}", name=f"CC{k}",
                           addr_space="Shared") for k in range(3)]
            DTB = [dp.tile([ACCN + 64, 128], f16, tag=f"DT{k}", name=f"DT{k}")
                   for k in range(4)]

            # const rows + zero-init of DSTTABs
            nc.sync.dma_start(out=ACCD[ACC_CONST_MID:ACC_CONST_MID + 2, :],
                              in_=din["accconst"].ap())
            zeros = pp.tile([128, ACCN], f16, tag="zeros")
            nc.vector.memset(zeros[:], 0.0)
            ones = pp.tile([128, 512], f16, tag="ones")
            nc.vector.memset(ones[:], 1.0)
            for k in range(4):
                nc.sync.dma_start(
                    out=DTB[k][0:ACCN, :].rearrange("(a p) w -> p a w", p=128),
                    in_=zeros[:].rearrange("p (a w) -> p a w", w=128))

            # ------------------------------------------------------------------
            def build_dst0():
                """DSTTAB[0] rows from raw node features (own block)."""
                dstg = wp.tile([128, NT_OWN, 128], f16, tag="dstg", bufs=1,
                               name="dstg0")
                for nt in range(NT_OWN):
                    pd = pmain.tile([128, WB], f32, space="PSUM", tag="pmain")
                    nc.tensor.matmul(out=pd[:, 0:128],
                                     lhsT=x0own[:, nt * 128:(nt + 1) * 128],
                                     rhs=wd[0][:, :], start=True, stop=True)
                    if nt % 2 == 0:
                        nc.vector.tensor_copy(out=dstg[:, nt, :], in_=pd[:, 0:128])
                    else:
                        nc.scalar.copy(out=dstg[:, nt, :], in_=pd[:, 0:128])
                nc.gpsimd.dma_scatter_add(
                    out_ap=DTB[0][:, :], in_ap=dstg[:], idxs_ap=dstscat[:],
                    num_idxs=NPAD, num_idxs_reg=NPAD, elem_size=128)

            # ------------------------------------------------------------------
            def build_B(L):
                """Full-table node matmul: B[L] rows for all NTOT nodes."""
                WBL = WE_MID if L < 3 else WE_OUT
                for blk in range(C):
                    if L == 0:
                        xb = wp.tile([F_IN, NPAD], f16, tag="xblk")
                        nc.sync.dma_start(out=xb[:], in_=din["x0T"].ap()[:, blk * NPAD:(blk + 1) * NPAD])
                        kn = F_IN
                    else:
                        xb = wp.tile([F + 1, NPAD], f16, tag="xblk", name="xblkm")
                        nc.sync.dma_start(out=xb[:], in_=CCT[L - 1][blk, :, :])
                        kn = F + 1
                    for j0 in range(0, NT_OWN, 4):
                        nb = min(4, NT_OWN - j0)
                        bs = wp.tile([128, 4, WE_MID], f16, tag="bstage")
                        for dj in range(nb):
                            j = j0 + dj
                            pb = pmain.tile([128, WB], f32, space="PSUM", tag="pmain")
                            nc.tensor.matmul(out=pb[:, 0:WBL],
                                             lhsT=xb[:kn, j * 128:(j + 1) * 128],
                                             rhs=wn[L][:kn, 0:WBL],
                                             start=True, stop=True)
                            if dj % 2 == 0:
                                nc.vector.tensor_copy(out=bs[:, dj, 0:WBL], in_=pb[:, 0:WBL])
                            else:
                                nc.scalar.copy(out=bs[:, dj, 0:WBL], in_=pb[:, 0:WBL])
                        nt = blk * NT_OWN + j0
                        nc.sync.dma_start(
                            out=B[L][nt * 128:(nt + nb) * 128, 0:WBL].rearrange(
                                "(b p) w -> p b w", p=128),
                            in_=bs[:, 0:nb, 0:WBL])

            # ------------------------------------------------------------------
            def edge_phase(L):
                mid = L < 3
                WBL = WB if mid else WB3      # gather row stride
                WE = WE_MID if mid else WE_OUT
                Hk = H if mid else 1
                MQ = 260 if mid else 65       # interleaved msg+den cols
                WAk = WA if mid else WA3      # acc row content
                SS = 324 if mid else 65       # col of summed scores
                DOFF = 260 if mid else 0      # psum col where dst-mm lands
                DW = 68 if mid else 66        # dst row cols used
                esrc = [None, EA, EB, EA][L]
                edst = [EA, EB, EA, None][L]

                for c in range(NCHUNK):
                    cs, ce = c * CH * 128, (c + 1) * CH * 128
                    gb = wp.tile([128, CH, WBL], f16, tag="gb", name="gb")
                    nc.gpsimd.dma_gather(
                        out_ap=gb[:], in_ap=B[L][:, :],
                        idxs_ap=srcidx[:, c * (CH * 8):(c + 1) * (CH * 8)],
                        num_idxs=CH * 128, num_idxs_reg=CH * 128, elem_size=WBL)
                    dr = wp.tile([SEG, CH, 128], f16, tag="dr", name="dr")
                    nc.sync.dma_start(
                        out=dr[:],
                        in_=DTB[L][c * CH * SEG:(c + 1) * CH * SEG, :].rearrange(
                            "(t s) w -> s t w", s=SEG))
                    if L == 0:
                        ech = wp.tile([E_IN + 1, CH * 128], f16, tag="ech0")
                        nc.sync.dma_start(out=ech[:], in_=din["e0T"].ap()[:, cs:ce])
                        ke = E_IN + 1
                    else:
                        ech = wp.tile([F, CH * 128], f16, tag="ech")
                        nc.sync.dma_start(out=ech[:], in_=esrc[0:F, cs:ce])
                        ke = F

                    tch = wp.tile([128, CH, WE], f16, tag="tch")
                    for tl in range(CH):
                        t = c * CH + tl
                        pm = pmain.tile([128, WB], f32, space="PSUM", tag="pmain")
                        nc.tensor.matmul(out=pm[:, 0:WE],
                                         lhsT=ech[:ke, tl * 128:(tl + 1) * 128],
                                         rhs=we[L][:ke, 0:WE], start=True,
                                         stop=False, skip_group_check=True)
                        nc.tensor.matmul(out=pm[:, DOFF:DOFF + DW],
                                         lhsT=sohT[:, t * 128:(t + 1) * 128],
                                         rhs=dr[:, tl, 0:DW], start=False,
                                         stop=True, skip_group_check=True)
                        nc.vector.tensor_tensor(out=tch[:, tl, :], in0=pm[:, 0:WE],
                                                in1=gb[:, tl, 0:WE], op=AOP.add)

                    # ---- batched ops over the whole chunk (all SBUF, compact out)
                    mwm = wp.tile([128, CH, MQ], f16, tag="mwm")
                    if mid:
                        lk = wp3.tile([128, CH, H], f16, tag="lk")
                        nc.scalar.mul(lk[:, :, 0:Hk], tch[:, :, SS:SS + Hk], NEG_SLOPE)
                        lm = wp3.tile([128, CH, H], f16, tag="lm")
                        nc.vector.tensor_tensor(out=lm[:, :, 0:Hk],
                                                in0=tch[:, :, SS:SS + Hk],
                                                in1=lk[:, :, 0:Hk], op=AOP.max)
                        ex = wp3.tile([128, CH, H], f16, tag="ex")
                        nc.scalar.activation(ex[:, :, 0:Hk], lm[:, :, 0:Hk], AF.Exp)
                        nc.vector.tensor_tensor(
                            out=mwm[:].rearrange("p c (h q) -> p c h q", h=Hk),
                            in0=tch[:, :, 0:MQ].rearrange("p c (h q) -> p c h q", h=Hk),
                            in1=ex[:, :, 0:Hk].unsqueeze(3).to_broadcast(
                                [128, CH, Hk, F + 1]),
                            op=AOP.mult)
                        eech = wp.tile([128, CH, F], f16, tag="eech")
                        nc.scalar.activation(eech[:], tch[:, :, 260:324], AF.Exp)
                        eem = wp.tile([128, CH, F], f16, tag="eem")
                        nc.vector.tensor_tensor(
                            out=eem[:], in0=eech[:],
                            in1=ones[:, 0:CH * F].rearrange("p (c f) -> p c f", f=F),
                            op=AOP.subtract)
                        rrch = wp.tile([128, CH, F], f16, tag="rrch")
                        nc.vector.tensor_tensor(
                            out=rrch[:], in0=tch[:, :, 260:324],
                            in1=zeros[:, 0:CH * F].rearrange("p (c f) -> p c f", f=F),
                            op=AOP.max)
                        ench = wp.tile([128, CH, F], f16, tag="ench")
                        nc.vector.tensor_tensor(out=ench[:], in0=eem[:], in1=rrch[:],
                                                op=AOP.min)
                        est = wp.tile([128, CH // 2, 128], f16, tag="est")
                    else:
                        # full-width leaky+exp (score col 65; other cols unused)
                        lk = wp3.tile([128, CH, WE_OUT], f16, tag="lk3", name="lk3")
                        nc.scalar.mul(lk[:], tch[:], NEG_SLOPE)
                        lm = wp3.tile([128, CH, WE_OUT], f16, tag="lm3", name="lm3")
                        nc.vector.tensor_tensor(out=lm[:], in0=tch[:], in1=lk[:],
                                                op=AOP.max)
                        ex = wp3.tile([128, CH, WE_OUT], f16, tag="ex3", name="ex3")
                        nc.scalar.activation(ex[:], lm[:], AF.Exp)
                        nc.vector.tensor_tensor(
                            out=mwm[:],
                            in0=tch[:, :, 0:MQ],
                            in1=ex[:, :, 65:66].to_broadcast([128, CH, MQ]),
                            op=AOP.mult)

                    acst = wp.tile([64, 4, WA], f32, tag="acst")
                    for g in range(4):
                        ps = pseg.tile([64, WA], f32, space="PSUM", tag="pseg")
                        for q in range(2):
                            tl = g * 2 + q
                            t = c * CH + tl
                            nc.tensor.matmul(out=ps[q * SEG:(q + 1) * SEG, 0:WAk],
                                             lhsT=soh[:, t * SEG:(t + 1) * SEG],
                                             rhs=mwm[:, tl, 0:WAk], start=True,
                                             stop=True, skip_group_check=True)
                        if g % 2 == 0:
                            nc.vector.tensor_copy(out=acst[:, g, 0:WAk], in_=ps[:, 0:WAk])
                        else:
                            nc.scalar.copy(out=acst[:, g, 0:WAk], in_=ps[:, 0:WAk])
                    if mid:
                        for pr2 in range(CH // 2):
                            pt = ptr.tile([128, 128], f16, space="PSUM", tag="ptr")
                            nc.tensor.transpose(
                                out=pt[:],
                                in_=ench[:, 2 * pr2:2 * pr2 + 2, :].rearrange(
                                    "p c f -> p (c f)"),
                                identity=ident[:])
                            if pr2 % 2 == 0:
                                nc.vector.tensor_copy(out=est[:, pr2, :], in_=pt[:])
                            else:
                                nc.scalar.copy(out=est[:, pr2, :], in_=pt[:])
                    nc.sync.dma_start(
                        out=ACCD[c * CH * SEG:(c + 1) * CH * SEG, 0:WAk].rearrange(
                            "(g p) w -> p g w", p=64),
                        in_=acst[:, :, 0:WAk])
                    if mid:
                        nc.sync.dma_start(
                            out=edst[0:F, cs:ce].rearrange(
                                "f (i j p) -> f j i p", j=2, p=128)[:, 0, :, :],
                            in_=est[0:F, :, :])
                        nc.sync.dma_start(
                            out=edst[0:F, cs:ce].rearrange(
                                "f (i j p) -> f j i p", j=2, p=128)[:, 1, :, :],
                            in_=est[F:128, :, :])

            # ------------------------------------------------------------------
            def node_phase(L):
                if L < 3:
                    gn = wp.tile([128, NT_OWN, 320], f32, tag="gn", bufs=1)
                    nc.gpsimd.dma_gather(
                        out_ap=gn[:, 0:8, :], in_ap=ACCD[:, :], idxs_ap=accidx_m[:, 0:64],
                        num_idxs=1024, num_idxs_reg=1024, elem_size=320)
                    nc.gpsimd.dma_gather(
                        out_ap=gn[:, 8:NT_OWN, :], in_ap=ACCD[:, :], idxs_ap=accidx_m[:, 64:88],
                        num_idxs=NPAD - 1024, num_idxs_reg=NPAD - 1024, elem_size=320)
                    xstg = wp.tile([F + 1, NT_OWN, 128], f16, tag="xstg", bufs=1)
                    nc.sync.dma_start(out=xstg[F:F + 1, :, :],
                                      in_=din["e_ones"].ap()[:, 0:NPAD])
                    dstg = wp.tile([128, NT_OWN, 128], f16, tag="dstg", bufs=1)
                    for nt in range(NT_OWN):
                        rec = wp3.tile([128, H], f32, tag="rec")
                        nc.vector.reciprocal(
                            out=rec[:],
                            in_=gn[:, nt, 0:260].rearrange(
                                "p (h q) -> p h q", q=F + 1)[:, :, F])
                        pr = wp3.tile([128, 256], f32, tag="pr")
                        nc.vector.tensor_tensor(
                            out=pr[:].rearrange("p (h f) -> p h f", h=H),
                            in0=gn[:, nt, 0:260].rearrange(
                                "p (h q) -> p h q", q=F + 1)[:, :, 0:F],
                            in1=rec[:].unsqueeze(2).to_broadcast([128, H, F]),
                            op=AOP.mult)
                        xo = wp3.tile([128, F], f32, tag="xo")
                        nc.vector.tensor_reduce(
                            out=xo[:], in_=pr[:].rearrange("p (h f) -> p f h", h=H),
                            axis=mybir.AxisListType.X, op=AOP.add)
                        # ELU(x) = min(exp(x)-1, max(x, 0))
                        xe = wp3.tile([128, F], f16, tag="ee")
                        nc.scalar.activation(xe[:], xo[:], AF.Exp)
                        xem = wp3.tile([128, F], f16, tag="eem1")
                        nc.vector.tensor_tensor(out=xem[:], in0=xe[:],
                                                in1=ones[:, 0:F], op=AOP.subtract)
                        xr = wp3.tile([128, F], f16, tag="rr")
                        nc.vector.tensor_tensor(out=xr[:], in0=xo[:],
                                                in1=zeros[:, 0:F], op=AOP.max)
                        xs = wp3.tile([128, F], f16, tag="en")
                        nc.vector.tensor_tensor(out=xs[:], in0=xem[:], in1=xr[:], op=AOP.min)
                        pt = ptr.tile([128, 128], f16, space="PSUM", tag="ptr")
                        nc.tensor.transpose(out=pt[0:F, :], in_=xs[:], identity=ident[:])
                        if nt % 2 == 0:
                            nc.vector.tensor_copy(out=xstg[0:F, nt, :], in_=pt[0:F, :])
                        else:
                            nc.scalar.copy(out=xstg[0:F, nt, :], in_=pt[0:F, :])
                        # next layer's dst-table rows (node-major)
                        pd = pmain.tile([128, WB], f32, space="PSUM", tag="pmain")
                        nc.tensor.matmul(out=pd[:, 0:128],
                                         lhsT=xstg[:, nt, :], rhs=wd[L + 1][:, :],
                                         start=True, stop=True)
                        if nt % 2 == 0:
                            nc.scalar.copy(out=dstg[:, nt, :], in_=pd[:, 0:128])
                        else:
                            nc.vector.tensor_copy(out=dstg[:, nt, :], in_=pd[:, 0:128])
                    nc.sync.dma_start(out=AGIN[:, :].rearrange("f (t p) -> f t p", p=128),
                                      in_=xstg[:])
                    nc.gpsimd.dma_scatter_add(
                        out_ap=DTB[L + 1][:, :], in_ap=dstg[:], idxs_ap=dstscat[:],
                        num_idxs=NPAD, num_idxs_reg=NPAD, elem_size=128)
                    nc.gpsimd.collective_compute(
                        "AllGather", AOP.bypass,
                        replica_groups=[list(range(C))],
                        ins=[AGIN[:]], outs=[CCT[L][:]])
                    build_B(L + 1)
                else:
                    gn = wp.tile([128, NT_OWN, 128], f32, tag="gn", name="gn3", bufs=1)
                    nc.gpsimd.dma_gather(
                        out_ap=gn[:, 0:8, :], in_ap=ACCD[:, 0:128], idxs_ap=accidx_o[:, 0:64],
                        num_idxs=1024, num_idxs_reg=1024, elem_size=128, elem_step=320)
                    nc.gpsimd.dma_gather(
                        out_ap=gn[:, 8:NT_OWN, :], in_ap=ACCD[:, 0:128], idxs_ap=accidx_o[:, 64:88],
                        num_idxs=NPAD - 1024, num_idxs_reg=NPAD - 1024, elem_size=128, elem_step=320)
                    pg = pgp.tile([8, OUT], f32, space="PSUM", tag="pg")
                    for nt in range(NT_OWN):
                        rec = wp3.tile([128, H], f32, tag="rec")
                        nc.vector.reciprocal(out=rec[:, 0:1], in_=gn[:, nt, 64:65])
                        nod = wp3.tile([128, 256], f32, tag="pr", name="nod")
                        nc.vector.tensor_tensor(out=nod[:, 0:OUT], in0=gn[:, nt, 0:OUT],
                                                in1=rec[:, 0:1].to_broadcast([128, OUT]),
                                                op=AOP.mult)
                        nc.tensor.matmul(out=pg[:], lhsT=g1h[:, nt * 8:(nt + 1) * 8],
                                         rhs=nod[:, 0:OUT], start=(nt == 0),
                                         stop=(nt == NT_OWN - 1), skip_group_check=True)
                    og = wp3.tile([8, OUT], f32, tag="og")
                    nc.vector.tensor_copy(out=og[:], in_=pg[:])
                    nc.sync.dma_start(out=out_t.ap(), in_=og[:])

            # ------------------------------------------------------------------
            stage = os.environ.get("KERNEL_STAGE", "full")
            og0 = wp3.tile([8, OUT], f32, tag="og", name="og0")
            nc.vector.memset(og0[:], 0.0)
            nc.sync.dma_start(out=out_t.ap(), in_=og0[:])
            if stage == "full":
                build_dst0()
                build_B(0)
                for L in range(4):
                    edge_phase(L)
                    node_phase(L)
            else:
                n = int(stage)  # 1=B0+dst0, 2=+edge0, 3=+node0, 4=+edge1, ...
                step = 0
                build_dst0()
                build_B(0)
                step += 1
                for L in range(4):
                    if step >= n:
                        break
                    edge_phase(L)
                    step += 1
                    if step >= n:
                        break
                    node_phase(L)
                    step += 1

    nc.compile()
    return nc


def _get_program():
    if "nc" not in _PROGRAM_CACHE:
        _PROGRAM_CACHE["nc"] = _build_program()
    return _PROGRAM_CACHE["nc"]


def kernel(**inputs):
    from concourse.bass_utils import run_bass_kernel_spmd

    nc = _get_program()
    in_maps = _preprocess(inputs)
    trace = bool(int(os.environ.get("KERNEL_TRACE", "0")))
    res = run_bass_kernel_spmd(nc, in_maps, core_ids=list(range(C)), trace=trace)
    _PROGRAM_CACHE["last_result"] = res
    out = np.concatenate([np.asarray(res.results[c]["out"]) for c in range(C)], axis=0)
    return out.astype(np.float32)
